# revision 42
# baseline (speedup 1.0000x reference)
"""Masked mean-pooling (nn_MaskedPooling) Trainium2 Bass kernel.

Reference semantics (jax):
    keep   = (~mask).astype(f32)               # [B, T]
    denom  = keep.sum(axis=1)                  # [B]
    out    = einsum('btd,bt->bd', x, keep) / denom[:, None]

Shapes: x [32, 4096, 512] f32, mask [32, 4096] bool -> out [32, 512] f32.

Strategy (data-parallel over batch, 8 NeuronCores, 4 examples/core):

RAGGED GATHER (default, MP_IMPL=gather):
  ~50% of the rows are masked out, so instead of streaming all of x we
  dma_gather only the kept rows (2 KiB each) out of HBM -- half the
  memory traffic, which is the roofline for this kernel.
  * Host side (cheap, mask-only): compact the kept t-indices per
    example, concatenate the core's 4 examples into one slot list,
    pad to a multiple of 128 with weight-0 dummy slots.  Slot s lands
    in SBUF partition s%128, chunk s//128 (dma_gather's layout).
  * The reduction is one long PSUM accumulation: for chunk c the
    stationary operand is a host-built [128, 4] weight matrix W with
    W[p, c, j] = 1/count_j if slot c*128+p belongs to example j else 0.
    This bakes the mean divide into the matmul AND makes the program
    shape independent of the per-core ragged structure (all cores run
    the same NEFF with different W/idx data).
  * f32r matmuls (single-pass fp32, PE 1 cyc/row) keep the PE far off
    the critical path; the gather DMA stream is the bottleneck.

DENSE fallback (MP_IMPL=dense): previous implementation -- stream all
of x, masked sum via [128,1]-stationary matmuls per T-chunk.
"""

import os
from contextlib import ExitStack

import numpy as np

import concourse.bass as bass
import concourse.mybir as mybir
import concourse.tile as tile
from concourse import bacc, bass_utils, library_config

B, T, D = 32, 4096, 512
N_CORES = 8
BS = B // N_CORES  # examples per core
P = 128  # SBUF partitions
NCHUNK = T // P  # T-chunks per example (32)

IMPL = os.environ.get("MP_IMPL", "gather")

# --- gather tunables ---
G_CHUNKS_PER_TILE = int(os.environ.get("MP_G_CPT", "8"))  # 1024 rows / 2 MiB
G_X_BUFS = int(os.environ.get("MP_G_XBUFS", "5"))

# --- dense tunables ---
CHUNKS_PER_TILE = int(os.environ.get("MP_CHUNKS_PER_TILE", "16"))
X_BUFS = int(os.environ.get("MP_X_BUFS", "5"))
MM_DTYPE = os.environ.get("MP_MM_DTYPE", "f32r")
N_DMA_ENGINES = int(os.environ.get("MP_DMA_ENGINES", "0"))

F32R = mybir.dt.float32r


# ---------------------------------------------------------------------------
# Ragged gather implementation
# ---------------------------------------------------------------------------


def prep_gather_inputs(x: np.ndarray, mask: np.ndarray):
    """Host-side (mask-only) prep: per-core compacted indices + weights.

    Returns (in_maps, nchunks, ntiles_chunks) where ntiles_chunks is the
    per-gather chunk count list (uniform across cores so one program fits
    all).
    """
    keep = ~mask  # [B, T] bool
    cpt = G_CHUNKS_PER_TILE

    # Per-core slot lists (global row ids into the core's flat [BS*T, D] x).
    core_slots = []
    core_examples = []  # per core: slot -> local example id
    core_counts = []
    for i in range(N_CORES):
        slots = []
        exids = []
        counts = []
        for j in range(BS):
            b = i * BS + j
            idx = np.flatnonzero(keep[b])
            counts.append(len(idx))
            slots.append(idx.astype(np.int32) + j * T)
            exids.append(np.full(len(idx), j, dtype=np.int32))
        core_slots.append(np.concatenate(slots))
        core_examples.append(np.concatenate(exids))
        core_counts.append(counts)

    nchunks = max((len(s) + P - 1) // P for s in core_slots)
    ntiles = (nchunks + cpt - 1) // cpt
    tile_chunks = [cpt] * (ntiles - 1) + [nchunks - cpt * (ntiles - 1)]
    nslots = nchunks * P

    in_maps = []
    for i in range(N_CORES):
        slots = core_slots[i]
        exids = core_examples[i]
        n = len(slots)
        pad = nslots - n
        slots_p = np.concatenate([slots, np.zeros(pad, dtype=np.int32)])

        # weights [128, nchunks, BS] f32
        wts = np.zeros((P, nchunks, BS), dtype=np.float32)
        s = np.arange(n)
        wts[s % P, s // P, exids] = 1.0 / np.asarray(core_counts[i], np.float32)[exids]

        # indices, wrapped per gather call: within a call of m idxs, idx k
        # sits at partition k%16, column k//16; replicated across the 8
        # groups of 16 partitions.  One [128, nslots//16] tensor, tiles
        # concatenated along the free dim.
        idx_cols = []
        pos = 0
        for tc in tile_chunks:
            m = tc * P
            seg = slots_p[pos : pos + m].astype(np.int16)
            idx_cols.append(seg.reshape(m // 16, 16).T)  # [16, m/16]
            pos += m
        idx16 = np.concatenate(idx_cols, axis=1)  # [16, nslots/16]
        idx128 = np.tile(idx16, (P // 16, 1))  # [128, nslots/16]

        in_maps.append(
            {
                "x": None,  # filled by caller (needs x slice)
                "idx": np.ascontiguousarray(idx128),
                "wts": np.ascontiguousarray(wts),
            }
        )
    return in_maps, nchunks, tile_chunks


def build_gather_bass(nchunks, tile_chunks, x_bufs=None, n_cores=N_CORES):
    if x_bufs is None:
        x_bufs = G_X_BUFS
    nslots = nchunks * P
    nc = bacc.Bacc(
        trn_type="TRN2",
        target_bir_lowering=False,
        debug=False,
        num_devices=n_cores,
    )
    x = nc.dram_tensor("x", [BS * T, D], F32R, kind="ExternalInput").ap()
    idx = nc.dram_tensor("idx", [P, nslots // 16], mybir.dt.int16, kind="ExternalInput").ap()
    wts = nc.dram_tensor("wts", [P, nchunks, BS], F32R, kind="ExternalInput").ap()
    out = nc.dram_tensor("out", [BS, D], mybir.dt.float32, kind="ExternalOutput").ap()

    with tile.TileContext(nc) as tc, ExitStack() as ctx:
        singles = ctx.enter_context(tc.tile_pool(name="singles", bufs=1))
        xpool = ctx.enter_context(tc.tile_pool(name="xpool", bufs=x_bufs))
        psum = ctx.enter_context(tc.tile_pool(name="psum", bufs=1, space="PSUM"))

        idx_sb = singles.tile([P, nslots // 16], mybir.dt.int16)
        nc.sync.dma_start(out=idx_sb, in_=idx)
        wts_sb = singles.tile([P, nchunks, BS], F32R)
        nc.sync.dma_start(out=wts_sb, in_=wts)

        acc = psum.tile([BS, D], mybir.dt.float32)

        c0 = 0  # chunk cursor
        s16 = 0  # idx column cursor
        for tcnt in tile_chunks:
            m = tcnt * P
            xt = xpool.tile([P, tcnt, D], F32R, tag="xt")
            nc.gpsimd.dma_gather(
                xt[:],
                x,
                idx_sb[:, s16 : s16 + m // 16],
                m,
                m,
                D,
            )
            for k in range(tcnt):
                c = c0 + k
                nc.tensor.matmul(
                    acc,
                    wts_sb[:, c, :],
                    xt[:, k, :],
                    start=(c == 0),
                    stop=(c == nchunks - 1),
                )
            c0 += tcnt
            s16 += m // 16

        o_sb = singles.tile([BS, D], mybir.dt.float32)
        nc.vector.tensor_copy(out=o_sb, in_=acc)
        nc.sync.dma_start(out=out, in_=o_sb)

    nc.finalize()
    return nc


def prepare_gather(x: np.ndarray, mask: np.ndarray):
    """Full prep for the gather impl: (nc, in_maps)."""
    in_maps, nchunks, tile_chunks = prep_gather_inputs(x, mask)
    for i in range(N_CORES):
        in_maps[i]["x"] = np.ascontiguousarray(x[i * BS : (i + 1) * BS]).reshape(
            BS * T, D
        )
    nc = build_gather_bass(nchunks, tile_chunks)
    return nc, in_maps


# ---------------------------------------------------------------------------
# Run-packed gather: cover each run of consecutive kept rows with 4 KiB
# pair-descriptors (elem_size=2 rows, elem_step=1 row -- overlapping source
# AP) plus 2 KiB single-descriptors for odd-run tails.  Exact HBM traffic
# (kept bytes only) with ~1.5x fewer descriptors than row-gather, which
# matters because Q7 SWDGE descriptor generation (~9.4 ns/idx) is the
# critical path of the row-gather kernel.
#
# Weight layout (stationary operands, [128, 2*CP + CS, BS] f32):
#   pair chunk c: col 2c weights the first row, col 2c+1 the second row.
#   single chunk c: col 2*CP + c.
# All matmuls accumulate into one [BS, D] PSUM chain; pad slots (dup row 0)
# carry weight 0.
# ---------------------------------------------------------------------------


GRANS = [4, 2, 1]  # rows per descriptor, packed greedily per run
G_CPT = {4: 4, 2: 8, 1: 8}  # chunks per gather tile
_bufs = os.environ.get("MP_RP_BUFS", "1,2,3").split(",")
G_BUFS = {4: int(_bufs[0]), 2: int(_bufs[1]), 1: int(_bufs[2])}
# The head (128*PREFIX_CHUNKS rows) of the first PREFIX_EXAMPLES examples of
# each core is read with plain HWDGE dma_starts + keep-weighted matmuls
# instead of gathers.  Those DMAs need no Q7 involvement, so they stream
# during the ~10us mlp library overlay load that blocks the first
# dma_gather -- free work in an otherwise idle window -- and they shrink
# the gather descriptor count.
PREFIX_CHUNKS = int(os.environ.get("MP_RP_PREFIX", "14"))
PREFIX_EXAMPLES = int(os.environ.get("MP_RP_PREFIX_EX", "1"))
# [128,1]-offset indirect_dma_start calls issued before the gathers.
# Measured NET NEGATIVE (default 0): the mlp IRAM overlay load blocks the
# whole Pool engine, so these cannot actually run during the overlay
# window -- they just serialize with the gathers (~1.1us per call).
IND_CALLS = int(os.environ.get("MP_RP_IND", "0"))


def _runs_pack(keep_row):
    """Greedy {4,2,1} run packing -> {g: start rows}, exact cover of kept."""
    t = np.flatnonzero(keep_row)
    out = {g: np.empty(0, np.int32) for g in GRANS}
    if len(t) == 0:
        return out
    new_run = np.ones(len(t), dtype=bool)
    new_run[1:] = np.diff(t) > 1
    run_id = np.cumsum(new_run) - 1
    run_start_pos = np.flatnonzero(new_run)
    pos = np.arange(len(t)) - run_start_pos[run_id]
    rl = np.bincount(run_id)[run_id]
    out[4] = t[(pos % 4 == 0) & (pos + 4 <= rl)].astype(np.int32)
    out[2] = t[(pos % 4 == 0) & (pos + 4 > rl) & (pos + 2 <= rl)].astype(np.int32)
    out[1] = t[(pos + 1 == rl) & (pos % 2 == 0)].astype(np.int32)
    return out


def _tile_split(n, cpt, small_last=False):
    """Chunk-tile sizes: full tiles first, then remainder (plus a final
    1-chunk tile when small_last, to shrink the end-of-pipeline drain)."""
    if n <= 0:
        return []
    if small_last:
        if n == 1:
            return [1]
        tiles = _tile_split(n - 1, cpt)
        tiles.append(1)
        return tiles
    tiles = [cpt] * (n // cpt)
    if n % cpt:
        tiles.append(n % cpt)
    return tiles


def _wrap16(vals):
    """[m] int -> [128, m/16] int16 wrapped (j -> partition j%16, col j//16)."""
    m = len(vals)
    w = vals.astype(np.int16).reshape(m // 16, 16).T
    return np.tile(w, (P // 16, 1))


def _balance_examples(keep):
    """Assign 4 examples per core, equalizing descriptor counts.  Returns
    assign[i] = list of 4 global example ids (largest-desc example first,
    so the dense prefix bites into the biggest one)."""
    ndesc = np.array(
        [sum(len(v) for v in _runs_pack(keep[b]).values()) for b in range(B)]
    )
    order = np.argsort(-ndesc)
    loads = [0.0] * N_CORES
    assign = [[] for _ in range(N_CORES)]
    for b in order:
        i = min(
            (i for i in range(N_CORES) if len(assign[i]) < BS),
            key=lambda i: loads[i],
        )
        assign[i].append(int(b))
        loads[i] += ndesc[b]
    return assign


def prep_runpack_inputs(x: np.ndarray, mask: np.ndarray):
    keep = ~mask
    nch_pre = PREFIX_CHUNKS
    rpre = nch_pre * P  # dense-prefix rows (from example 0 of each core)
    assert rpre <= T

    assign = _balance_examples(keep)

    core_slots = [dict() for _ in range(N_CORES)]  # core -> g -> starts
    core_exid = [dict() for _ in range(N_CORES)]
    core_counts = []
    for i in range(N_CORES):
        acc = {g: [] for g in GRANS}
        exa = {g: [] for g in GRANS}
        counts = []
        for j in range(BS):
            b = assign[i][j]
            krow = keep[b]
            counts.append(np.count_nonzero(krow))
            if j < PREFIX_EXAMPLES and rpre:
                krow = krow.copy()
                krow[:rpre] = False  # covered by the dense prefix
            packed = _runs_pack(krow)
            for g in GRANS:
                acc[g].append(packed[g] + j * T)
                exa[g].append(np.full(len(packed[g]), j, np.int32))
        for g in GRANS:
            core_slots[i][g] = np.concatenate(acc[g])
            core_exid[i][g] = np.concatenate(exa[g])
        core_counts.append(counts)

    # pull the first nind*128 singles out for the indirect-DMA prologue
    nind = min(IND_CALLS, min(len(core_slots[i][1]) for i in range(N_CORES)) // P)
    core_ioff = []
    core_iex = []
    for i in range(N_CORES):
        m = nind * P
        core_ioff.append(core_slots[i][1][:m].astype(np.int32))
        core_iex.append(core_exid[i][1][:m])
        core_slots[i][1] = core_slots[i][1][m:]
        core_exid[i][1] = core_exid[i][1][m:]

    # uniform chunk counts across cores (one program for all)
    CG = {
        g: max((len(core_slots[i][g]) + P - 1) // P for i in range(N_CORES))
        for g in GRANS
    }
    # program order: big descriptors first (they overfeed the SDMA engines,
    # singles underfeed -- this ordering drains the ring backlog by the end),
    # a 1-chunk singles tile last to shrink the final drain
    order = []
    for g in GRANS:
        for tc in _tile_split(CG[g], G_CPT[g], small_last=(g == 1)):
            order.append((g, tc))

    # weight columns:
    # [indirect calls][dense prefix chunks x examples][gran 4][gran 2][gran 1]
    woff = {}
    o = nind + nch_pre * PREFIX_EXAMPLES
    for g in GRANS:
        woff[g] = o
        o += g * CG[g]
    nwcols = o

    in_maps = []
    for i in range(N_CORES):
        inv = 1.0 / np.asarray(core_counts[i], np.float32)
        wts = np.zeros((P, nwcols, BS), dtype=np.float32)
        # indirect-call weights: call c row p -> col c
        s = np.arange(nind * P)
        wts[s % P, s // P, core_iex[i]] = inv[core_iex[i]]
        # dense prefix weights: row p*nch_pre + n of example j -> col nind+j*nch_pre+n
        if rpre:
            for j in range(PREFIX_EXAMPLES):
                kp = keep[assign[i][j], :rpre].reshape(P, nch_pre)
                wts[:, nind + j * nch_pre : nind + (j + 1) * nch_pre, j] = kp * inv[j]
        slots_p = {}
        for g in GRANS:
            slots = core_slots[i][g]
            ex = core_exid[i][g]
            n = len(slots)
            slots_p[g] = np.concatenate(
                [slots, np.zeros(CG[g] * P - n, dtype=np.int32)]
            )
            s = np.arange(n)
            for h in range(g):
                wts[s % P, woff[g] + g * (s // P) + h, ex] = inv[ex]

        # idx tensor: per-tile wrapped segments in program order
        segs = []
        cur = {g: 0 for g in GRANS}
        for g, tc in order:
            m = tc * P
            segs.append(_wrap16(slots_p[g][cur[g] * P : cur[g] * P + m]))
            cur[g] += tc
        idx128 = np.concatenate(segs, axis=1)

        im = {
            "x": None,
            "idx": np.ascontiguousarray(idx128),
            "wts": np.ascontiguousarray(wts),
        }
        if nind:
            im["ioff"] = np.ascontiguousarray(core_ioff[i].reshape(nind, P).T)
        in_maps.append(im)
    return in_maps, (CG, woff, order, nind), assign


def build_runpack_bass(meta, n_cores=N_CORES):
    CG, woff, order, nind = meta
    nch_pre = PREFIX_CHUNKS
    npre_ex = PREFIX_EXAMPLES
    nidxcols = sum(CG[g] for g in GRANS) * P // 16
    nwcols = nind + nch_pre * npre_ex + sum(g * CG[g] for g in GRANS)
    total_mm = nwcols  # one matmul per weight column

    nc = bacc.Bacc(
        trn_type="TRN2",
        target_bir_lowering=False,
        debug=False,
        num_devices=n_cores,
    )
    x = nc.dram_tensor("x", [BS * T, D], F32R, kind="ExternalInput").ap()
    idx = nc.dram_tensor("idx", [P, nidxcols], mybir.dt.int16, kind="ExternalInput").ap()
    wts = nc.dram_tensor("wts", [P, nwcols, BS], F32R, kind="ExternalInput").ap()
    out = nc.dram_tensor("out", [BS, D], mybir.dt.float32, kind="ExternalOutput").ap()
    ioff = None
    if nind:
        ioff = nc.dram_tensor(
            "ioff", [P, nind], mybir.dt.int32, kind="ExternalInput"
        ).ap()

    # overlapping views: row i -> g*D contiguous f32 starting at row i
    xview = {}
    for g in GRANS:
        v = x.copy()
        v.ap = type(x.ap)([[D, BS * T - (g - 1)], [1, g * D]])
        xview[g] = v

    with tile.TileContext(nc) as tc, ExitStack() as ctx:
        singles_pool = ctx.enter_context(tc.tile_pool(name="singles", bufs=1))
        pools = {
            g: ctx.enter_context(tc.tile_pool(name=f"pool{g}", bufs=G_BUFS[g]))
            for g in GRANS
            if CG[g]
        }
        psum = ctx.enter_context(tc.tile_pool(name="psum", bufs=1, space="PSUM"))

        # kick the mlp IRAM overlay load as early as possible -- the first
        # dma_gather blocks on it for ~10us
        nc.gpsimd.load_library(library_config.mlp)

        idx_sb = singles_pool.tile([P, nidxcols], mybir.dt.int16)
        nc.sync.dma_start(out=idx_sb, in_=idx)
        wts_sb = singles_pool.tile([P, nwcols, BS], F32R)
        nc.sync.dma_start(out=wts_sb, in_=wts)

        acc = psum.tile([BS, D], mybir.dt.float32)

        mm = 0  # matmul counter for start/stop flags
        icol = 0  # idx column cursor (16-wrapped units)
        cur = {g: 0 for g in GRANS}  # chunk cursors

        # indirect-DMA prologue: fetch 128 kept single-rows per call through
        # the mainline SWDGE path while the mlp overlay loads.  high_priority
        # pins these to the front of the Pool-engine schedule -- without it
        # the scheduler interleaves them between gathers where they are pure
        # serial cost.
        if nind:
            with tc.high_priority():
                ioff_sb = singles_pool.tile([P, nind], mybir.dt.int32)
                nc.sync.dma_start(out=ioff_sb, in_=ioff)
                indpool = ctx.enter_context(tc.tile_pool(name="indpool", bufs=3))
                for c in range(nind):
                    xt = indpool.tile([P, D], F32R, tag="xi")
                    nc.gpsimd.indirect_dma_start(
                        out=xt[:],
                        out_offset=None,
                        in_=x,
                        in_offset=bass.IndirectOffsetOnAxis(
                            ap=ioff_sb[:, c : c + 1], axis=0
                        ),
                    )
                    nc.tensor.matmul(
                        acc,
                        wts_sb[:, c, :],
                        xt[:],
                        start=(mm == 0),
                        stop=(mm == total_mm - 1),
                    )
                    mm += 1

        # dense prefix: HWDGE dma_starts (no Q7 involvement, and they must
        # not queue on the SWDGE ring where they would delay the mlp library
        # overlay load) + keep-weighted matmuls
        pre_eng = [nc.scalar, nc.sync]
        for j in range(npre_ex if nch_pre else 0):
            xpre = singles_pool.tile([P, nch_pre, D], F32R, tag=f"xpre{j}")
            pre_eng[j % 2].dma_start(
                out=xpre,
                in_=x[j * T : j * T + nch_pre * P].rearrange(
                    "(p n) d -> p n d", p=P
                ),
            )
            for n in range(nch_pre):
                nc.tensor.matmul(
                    acc,
                    wts_sb[:, nind + j * nch_pre + n, :],
                    xpre[:, n, :],
                    start=(mm == 0),
                    stop=(mm == total_mm - 1),
                )
                mm += 1

        for g, tc_n in order:
            m = tc_n * P
            xt = pools[g].tile([P, tc_n, g * D], F32R, tag=f"x{g}")
            nc.gpsimd.dma_gather(
                xt[:],
                xview[g],
                idx_sb[:, icol : icol + m // 16],
                m,
                m,
                g * D,
                elem_step=D,
            )
            for k in range(tc_n):
                c = cur[g] + k
                for h in range(g):
                    nc.tensor.matmul(
                        acc,
                        wts_sb[:, woff[g] + g * c + h, :],
                        xt[:, k, h * D : (h + 1) * D],
                        start=(mm == 0),
                        stop=(mm == total_mm - 1),
                    )
                    mm += 1
            cur[g] += tc_n
            icol += m // 16
        assert mm == total_mm

        o_sb = singles_pool.tile([BS, D], mybir.dt.float32)
        nc.vector.tensor_copy(out=o_sb, in_=acc)
        nc.sync.dma_start(out=out, in_=o_sb)

    nc.finalize()
    return nc


def prepare_runpack(x: np.ndarray, mask: np.ndarray):
    in_maps, meta, assign = prep_runpack_inputs(x, mask)
    for i in range(N_CORES):
        in_maps[i]["x"] = np.ascontiguousarray(x[assign[i]]).reshape(BS * T, D)
    nc = build_runpack_bass(meta)

    def unshard(results):
        out = np.empty((B, D), dtype=np.float32)
        for i in range(N_CORES):
            out[assign[i]] = results[i]["out"]
        return out

    return nc, in_maps, unshard


# ---------------------------------------------------------------------------
# Indirect-DMA implementation (mainline SWDGE dynamic AP instead of the
# dma_gather extended instruction -- different Q7 descriptor-gen path).
# Layout: slot (p, c) of the [128, NCOL, 512] gathered tensor reads row
# off[p, c]; within a tile of ct columns the flat slot order is p-major.
# ---------------------------------------------------------------------------


def prep_indirect_inputs(x: np.ndarray, mask: np.ndarray):
    keep = ~mask
    cpt = G_CHUNKS_PER_TILE

    core_slots = []
    core_examples = []
    core_counts = []
    for i in range(N_CORES):
        slots = []
        exids = []
        counts = []
        for j in range(BS):
            b = i * BS + j
            idx = np.flatnonzero(keep[b])
            counts.append(len(idx))
            slots.append(idx.astype(np.int32) + j * T)
            exids.append(np.full(len(idx), j, dtype=np.int32))
        core_slots.append(np.concatenate(slots))
        core_examples.append(np.concatenate(exids))
        core_counts.append(counts)

    ncol = max((len(s) + P - 1) // P for s in core_slots)
    ntiles = (ncol + cpt - 1) // cpt
    tile_cols = [cpt] * (ntiles - 1) + [ncol - cpt * (ntiles - 1)]
    nslots = ncol * P

    in_maps = []
    for i in range(N_CORES):
        slots = core_slots[i]
        exids = core_examples[i]
        n = len(slots)
        pad = nslots - n
        slots_p = np.concatenate([slots, np.zeros(pad, dtype=np.int32)])
        exids_p = np.concatenate([exids, np.zeros(pad, dtype=np.int32)])
        inv = 1.0 / np.asarray(core_counts[i], np.float32)

        off = np.zeros((P, ncol), dtype=np.int32)
        wts = np.zeros((P, ncol, BS), dtype=np.float32)
        pos = 0
        c0 = 0
        for ct in tile_cols:
            m = ct * P
            blk = slots_p[pos : pos + m].reshape(P, ct)
            off[:, c0 : c0 + ct] = blk
            eb = exids_p[pos : pos + m].reshape(P, ct)
            pp, cc = np.meshgrid(np.arange(P), np.arange(ct), indexing="ij")
            w = np.zeros((P, ct, BS), dtype=np.float32)
            valid = (pos + np.arange(m).reshape(P, ct)) < n
            w[pp, cc, eb] = np.where(valid, inv[eb], 0.0)
            wts[:, c0 : c0 + ct, :] = w
            pos += m
            c0 += ct

        in_maps.append(
            {
                "x": None,
                "off": np.ascontiguousarray(off),
                "wts": np.ascontiguousarray(wts),
            }
        )
    return in_maps, ncol, tile_cols


def build_indirect_bass(ncol, tile_cols, x_bufs=None, n_cores=N_CORES):
    if x_bufs is None:
        x_bufs = G_X_BUFS
    nc = bacc.Bacc(
        trn_type="TRN2",
        target_bir_lowering=False,
        debug=False,
        num_devices=n_cores,
    )
    x = nc.dram_tensor("x", [BS * T, D], F32R, kind="ExternalInput").ap()
    off = nc.dram_tensor("off", [P, ncol], mybir.dt.int32, kind="ExternalInput").ap()
    wts = nc.dram_tensor("wts", [P, ncol, BS], F32R, kind="ExternalInput").ap()
    out = nc.dram_tensor("out", [BS, D], mybir.dt.float32, kind="ExternalOutput").ap()

    with tile.TileContext(nc) as tc, ExitStack() as ctx:
        singles = ctx.enter_context(tc.tile_pool(name="singles", bufs=1))
        xpool = ctx.enter_context(tc.tile_pool(name="xpool", bufs=x_bufs))
        psum = ctx.enter_context(tc.tile_pool(name="psum", bufs=1, space="PSUM"))

        off_sb = singles.tile([P, ncol], mybir.dt.int32)
        nc.sync.dma_start(out=off_sb, in_=off)
        wts_sb = singles.tile([P, ncol, BS], F32R)
        nc.sync.dma_start(out=wts_sb, in_=wts)

        acc = psum.tile([BS, D], mybir.dt.float32)

        c0 = 0
        for ct in tile_cols:
            xt = xpool.tile([P, ct, D], F32R, tag="xt")
            nc.gpsimd.indirect_dma_start(
                out=xt[:],
                out_offset=None,
                in_=x,
                in_offset=bass.IndirectOffsetOnAxis(
                    ap=off_sb[:, c0 : c0 + ct],
                    axis=0,
                ),
            )
            for k in range(ct):
                c = c0 + k
                nc.tensor.matmul(
                    acc,
                    wts_sb[:, c, :],
                    xt[:, k, :],
                    start=(c == 0),
                    stop=(c == ncol - 1),
                )
            c0 += ct

        o_sb = singles.tile([BS, D], mybir.dt.float32)
        nc.vector.tensor_copy(out=o_sb, in_=acc)
        nc.sync.dma_start(out=out, in_=o_sb)

    nc.finalize()
    return nc


def prepare_indirect(x: np.ndarray, mask: np.ndarray):
    in_maps, ncol, tile_cols = prep_indirect_inputs(x, mask)
    for i in range(N_CORES):
        in_maps[i]["x"] = np.ascontiguousarray(x[i * BS : (i + 1) * BS]).reshape(
            BS * T, D
        )
    nc = build_indirect_bass(ncol, tile_cols)
    return nc, in_maps


# ---------------------------------------------------------------------------
# Dense fallback (previous implementation)
# ---------------------------------------------------------------------------


def build_bass(
    bs=BS,
    t=T,
    d=D,
    chunks_per_tile=CHUNKS_PER_TILE,
    x_bufs=X_BUFS,
    mm_dtype=MM_DTYPE,
    n_cores=N_CORES,
    n_dma_engines=N_DMA_ENGINES,
):
    nchunk = t // P
    assert t % P == 0 and nchunk % chunks_per_tile == 0
    nc = bacc.Bacc(
        trn_type="TRN2",
        target_bir_lowering=False,
        debug=False,
        num_devices=n_cores,
    )
    mmdt = mybir.dt.float32r if mm_dtype == "f32r" else mybir.dt.float32
    x = nc.dram_tensor("x", [bs, t, d], mmdt, kind="ExternalInput").ap()
    mask = nc.dram_tensor("mask", [bs, t], mybir.dt.uint8, kind="ExternalInput").ap()
    out = nc.dram_tensor("out", [bs, d], mybir.dt.float32, kind="ExternalOutput").ap()

    with tile.TileContext(nc) as tc, ExitStack() as ctx:
        singles = ctx.enter_context(tc.tile_pool(name="singles", bufs=1))
        xpool = ctx.enter_context(tc.tile_pool(name="xpool", bufs=x_bufs))
        tails = ctx.enter_context(tc.tile_pool(name="tails", bufs=4))
        psum = ctx.enter_context(tc.tile_pool(name="psum", bufs=1, space="PSUM"))
        accs = ctx.enter_context(tc.tile_pool(name="accs", bufs=4, space="PSUM"))

        jcols = bs * nchunk
        assert jcols <= 512

        ones = singles.tile([P, 1], mmdt)
        if mmdt == mybir.dt.float32r:
            ones_f32 = singles.tile([P, 1], mybir.dt.float32)
            nc.vector.memset(ones_f32, 1.0)
            nc.vector.tensor_copy(out=ones, in_=ones_f32)
        else:
            nc.vector.memset(ones, 1.0)

        m_u8 = singles.tile([P, bs, nchunk], mybir.dt.uint8)
        nc.sync.dma_start(out=m_u8, in_=mask.rearrange("b (p n) -> p b n", p=P))
        m_f = singles.tile([P, bs, nchunk], mybir.dt.float32)
        nc.vector.tensor_copy(out=m_f, in_=m_u8)
        keep = singles.tile([P, bs, nchunk], mmdt)
        nc.vector.tensor_scalar(
            out=keep,
            in0=m_f,
            scalar1=-1.0,
            scalar2=1.0,
            op0=mybir.AluOpType.mult,
            op1=mybir.AluOpType.add,
        )

        den_ps = psum.tile([1, bs, nchunk], mybir.dt.float32)
        nc.tensor.matmul(den_ps, ones, keep, start=True, stop=True)
        den = tails.tile([1, bs], mybir.dt.float32)
        nc.vector.tensor_reduce(
            out=den,
            in_=den_ps,
            axis=mybir.AxisListType.X,
            op=mybir.AluOpType.add,
        )
        rec = tails.tile([1, bs], mybir.dt.float32)
        nc.vector.reciprocal(rec, den)

        if n_dma_engines == 0:
            dma_engines = [nc.gpsimd]
            out_dma = nc.sync
        else:
            dma_engines = [nc.sync, nc.scalar][:n_dma_engines]
            out_dma = nc.gpsimd

        def segments(b):
            return [chunks_per_tile] * (nchunk // chunks_per_tile)

        dma_i = 0
        for b in range(bs):
            x_b = x[b].rearrange("(p n) d -> p n d", p=P)
            acc_ps = accs.tile([1, d], mybir.dt.float32)
            n0 = 0
            for seg in segments(b):
                x_tile = xpool.tile([P, seg, d], mmdt, tag="x_tile")
                dma_engines[dma_i % len(dma_engines)].dma_start(
                    out=x_tile,
                    in_=x_b[:, n0 : n0 + seg, :],
                )
                dma_i += 1
                for k in range(seg):
                    n = n0 + k
                    nc.tensor.matmul(
                        acc_ps,
                        keep[:, b, n : n + 1],
                        x_tile[:, k, :],
                        start=(n == 0),
                        stop=(n == nchunk - 1),
                    )
                n0 += seg
            o_sb = tails.tile([1, d], mybir.dt.float32)
            nc.vector.tensor_scalar_mul(o_sb, acc_ps, rec[0:1, b : b + 1])
            out_dma.dma_start(out=out[b : b + 1, :], in_=o_sb)

    nc.finalize()
    return nc


def prepare_dense(x: np.ndarray, mask: np.ndarray):
    nc = build_bass()
    mask_u8 = np.ascontiguousarray(mask).view(np.uint8)
    in_maps = [
        {
            "x": np.ascontiguousarray(x[i * BS : (i + 1) * BS]),
            "mask": np.ascontiguousarray(mask_u8[i * BS : (i + 1) * BS]),
        }
        for i in range(N_CORES)
    ]
    return nc, in_maps


def _concat_unshard(results):
    return np.concatenate([r["out"] for r in results], axis=0).astype(
        np.float32, copy=False
    )


def prepare(x: np.ndarray, mask: np.ndarray):
    """Returns (nc, in_maps, unshard) -- unshard maps per-core result dicts
    to the full [B, D] output."""
    if IMPL == "gather":
        return (*prepare_gather(x, mask), _concat_unshard)
    if IMPL == "indirect":
        return (*prepare_indirect(x, mask), _concat_unshard)
    if IMPL == "runpack":
        return prepare_runpack(x, mask)
    return (*prepare_dense(x, mask), _concat_unshard)


def kernel(x: np.ndarray, mask: np.ndarray) -> np.ndarray:
    assert x.shape == (B, T, D) and mask.shape == (B, T)
    nc, in_maps, unshard = prepare(x, mask)
    res = bass_utils.run_bass_kernel_spmd(nc, in_maps, core_ids=list(range(N_CORES)))
    return unshard(res.results).astype(np.float32, copy=False)


# revision 43
# speedup vs baseline: 1.0147x; 1.0147x over previous
"""Masked mean-pooling (nn_MaskedPooling) Trainium2 Bass kernel.

Reference semantics (jax):
    keep   = (~mask).astype(f32)               # [B, T]
    denom  = keep.sum(axis=1)                  # [B]
    out    = einsum('btd,bt->bd', x, keep) / denom[:, None]

Shapes: x [32, 4096, 512] f32, mask [32, 4096] bool -> out [32, 512] f32.

Strategy (data-parallel over batch, 8 NeuronCores, 4 examples/core):

RAGGED GATHER (default, MP_IMPL=gather):
  ~50% of the rows are masked out, so instead of streaming all of x we
  dma_gather only the kept rows (2 KiB each) out of HBM -- half the
  memory traffic, which is the roofline for this kernel.
  * Host side (cheap, mask-only): compact the kept t-indices per
    example, concatenate the core's 4 examples into one slot list,
    pad to a multiple of 128 with weight-0 dummy slots.  Slot s lands
    in SBUF partition s%128, chunk s//128 (dma_gather's layout).
  * The reduction is one long PSUM accumulation: for chunk c the
    stationary operand is a host-built [128, 4] weight matrix W with
    W[p, c, j] = 1/count_j if slot c*128+p belongs to example j else 0.
    This bakes the mean divide into the matmul AND makes the program
    shape independent of the per-core ragged structure (all cores run
    the same NEFF with different W/idx data).
  * f32r matmuls (single-pass fp32, PE 1 cyc/row) keep the PE far off
    the critical path; the gather DMA stream is the bottleneck.

DENSE fallback (MP_IMPL=dense): previous implementation -- stream all
of x, masked sum via [128,1]-stationary matmuls per T-chunk.
"""

import os
from contextlib import ExitStack

import numpy as np

import concourse.bass as bass
import concourse.mybir as mybir
import concourse.tile as tile
from concourse import bacc, bass_utils, library_config

B, T, D = 32, 4096, 512
N_CORES = 8
BS = B // N_CORES  # examples per core
P = 128  # SBUF partitions
NCHUNK = T // P  # T-chunks per example (32)

IMPL = os.environ.get("MP_IMPL", "gather")

# --- gather tunables ---
G_CHUNKS_PER_TILE = int(os.environ.get("MP_G_CPT", "8"))  # 1024 rows / 2 MiB
G_X_BUFS = int(os.environ.get("MP_G_XBUFS", "5"))

# --- dense tunables ---
CHUNKS_PER_TILE = int(os.environ.get("MP_CHUNKS_PER_TILE", "16"))
X_BUFS = int(os.environ.get("MP_X_BUFS", "5"))
MM_DTYPE = os.environ.get("MP_MM_DTYPE", "f32r")
N_DMA_ENGINES = int(os.environ.get("MP_DMA_ENGINES", "0"))

F32R = mybir.dt.float32r


# ---------------------------------------------------------------------------
# Ragged gather implementation
# ---------------------------------------------------------------------------


def prep_gather_inputs(x: np.ndarray, mask: np.ndarray):
    """Host-side (mask-only) prep: per-core compacted indices + weights.

    Returns (in_maps, nchunks, ntiles_chunks) where ntiles_chunks is the
    per-gather chunk count list (uniform across cores so one program fits
    all).
    """
    keep = ~mask  # [B, T] bool
    cpt = G_CHUNKS_PER_TILE

    # Per-core slot lists (global row ids into the core's flat [BS*T, D] x).
    core_slots = []
    core_examples = []  # per core: slot -> local example id
    core_counts = []
    for i in range(N_CORES):
        slots = []
        exids = []
        counts = []
        for j in range(BS):
            b = i * BS + j
            idx = np.flatnonzero(keep[b])
            counts.append(len(idx))
            slots.append(idx.astype(np.int32) + j * T)
            exids.append(np.full(len(idx), j, dtype=np.int32))
        core_slots.append(np.concatenate(slots))
        core_examples.append(np.concatenate(exids))
        core_counts.append(counts)

    nchunks = max((len(s) + P - 1) // P for s in core_slots)
    ntiles = (nchunks + cpt - 1) // cpt
    tile_chunks = [cpt] * (ntiles - 1) + [nchunks - cpt * (ntiles - 1)]
    nslots = nchunks * P

    in_maps = []
    for i in range(N_CORES):
        slots = core_slots[i]
        exids = core_examples[i]
        n = len(slots)
        pad = nslots - n
        slots_p = np.concatenate([slots, np.zeros(pad, dtype=np.int32)])

        # weights [128, nchunks, BS] f32
        wts = np.zeros((P, nchunks, BS), dtype=np.float32)
        s = np.arange(n)
        wts[s % P, s // P, exids] = 1.0 / np.asarray(core_counts[i], np.float32)[exids]

        # indices, wrapped per gather call: within a call of m idxs, idx k
        # sits at partition k%16, column k//16; replicated across the 8
        # groups of 16 partitions.  One [128, nslots//16] tensor, tiles
        # concatenated along the free dim.
        idx_cols = []
        pos = 0
        for tc in tile_chunks:
            m = tc * P
            seg = slots_p[pos : pos + m].astype(np.int16)
            idx_cols.append(seg.reshape(m // 16, 16).T)  # [16, m/16]
            pos += m
        idx16 = np.concatenate(idx_cols, axis=1)  # [16, nslots/16]
        idx128 = np.tile(idx16, (P // 16, 1))  # [128, nslots/16]

        in_maps.append(
            {
                "x": None,  # filled by caller (needs x slice)
                "idx": np.ascontiguousarray(idx128),
                "wts": np.ascontiguousarray(wts),
            }
        )
    return in_maps, nchunks, tile_chunks


def build_gather_bass(nchunks, tile_chunks, x_bufs=None, n_cores=N_CORES):
    if x_bufs is None:
        x_bufs = G_X_BUFS
    nslots = nchunks * P
    nc = bacc.Bacc(
        trn_type="TRN2",
        target_bir_lowering=False,
        debug=False,
        num_devices=n_cores,
    )
    x = nc.dram_tensor("x", [BS * T, D], F32R, kind="ExternalInput").ap()
    idx = nc.dram_tensor("idx", [P, nslots // 16], mybir.dt.int16, kind="ExternalInput").ap()
    wts = nc.dram_tensor("wts", [P, nchunks, BS], F32R, kind="ExternalInput").ap()
    out = nc.dram_tensor("out", [BS, D], mybir.dt.float32, kind="ExternalOutput").ap()

    with tile.TileContext(nc) as tc, ExitStack() as ctx:
        singles = ctx.enter_context(tc.tile_pool(name="singles", bufs=1))
        xpool = ctx.enter_context(tc.tile_pool(name="xpool", bufs=x_bufs))
        psum = ctx.enter_context(tc.tile_pool(name="psum", bufs=1, space="PSUM"))

        idx_sb = singles.tile([P, nslots // 16], mybir.dt.int16)
        nc.sync.dma_start(out=idx_sb, in_=idx)
        wts_sb = singles.tile([P, nchunks, BS], F32R)
        nc.sync.dma_start(out=wts_sb, in_=wts)

        acc = psum.tile([BS, D], mybir.dt.float32)

        c0 = 0  # chunk cursor
        s16 = 0  # idx column cursor
        for tcnt in tile_chunks:
            m = tcnt * P
            xt = xpool.tile([P, tcnt, D], F32R, tag="xt")
            nc.gpsimd.dma_gather(
                xt[:],
                x,
                idx_sb[:, s16 : s16 + m // 16],
                m,
                m,
                D,
            )
            for k in range(tcnt):
                c = c0 + k
                nc.tensor.matmul(
                    acc,
                    wts_sb[:, c, :],
                    xt[:, k, :],
                    start=(c == 0),
                    stop=(c == nchunks - 1),
                )
            c0 += tcnt
            s16 += m // 16

        o_sb = singles.tile([BS, D], mybir.dt.float32)
        nc.vector.tensor_copy(out=o_sb, in_=acc)
        nc.sync.dma_start(out=out, in_=o_sb)

    nc.finalize()
    return nc


def prepare_gather(x: np.ndarray, mask: np.ndarray):
    """Full prep for the gather impl: (nc, in_maps)."""
    in_maps, nchunks, tile_chunks = prep_gather_inputs(x, mask)
    for i in range(N_CORES):
        in_maps[i]["x"] = np.ascontiguousarray(x[i * BS : (i + 1) * BS]).reshape(
            BS * T, D
        )
    nc = build_gather_bass(nchunks, tile_chunks)
    return nc, in_maps


# ---------------------------------------------------------------------------
# Run-packed gather: cover each run of consecutive kept rows with 4 KiB
# pair-descriptors (elem_size=2 rows, elem_step=1 row -- overlapping source
# AP) plus 2 KiB single-descriptors for odd-run tails.  Exact HBM traffic
# (kept bytes only) with ~1.5x fewer descriptors than row-gather, which
# matters because Q7 SWDGE descriptor generation (~9.4 ns/idx) is the
# critical path of the row-gather kernel.
#
# Weight layout (stationary operands, [128, 2*CP + CS, BS] f32):
#   pair chunk c: col 2c weights the first row, col 2c+1 the second row.
#   single chunk c: col 2*CP + c.
# All matmuls accumulate into one [BS, D] PSUM chain; pad slots (dup row 0)
# carry weight 0.
# ---------------------------------------------------------------------------


GRANS = [4, 2, 1]  # rows per descriptor, packed greedily per run
G_CPT = {4: 4, 2: 8, 1: 8}  # chunks per gather tile
_bufs = os.environ.get("MP_RP_BUFS", "1,2,4").split(",")
G_BUFS = {4: int(_bufs[0]), 2: int(_bufs[1]), 1: int(_bufs[2])}
# The head (128*PREFIX_CHUNKS rows) of the first PREFIX_EXAMPLES examples of
# each core is read with plain HWDGE dma_starts + keep-weighted matmuls
# instead of gathers.  Those DMAs need no Q7 involvement, so they stream
# during the ~10us mlp library overlay load that blocks the first
# dma_gather -- free work in an otherwise idle window -- and they shrink
# the gather descriptor count.
PREFIX_CHUNKS = int(os.environ.get("MP_RP_PREFIX", "14"))
PREFIX_EXAMPLES = int(os.environ.get("MP_RP_PREFIX_EX", "1"))
# [128,1]-offset indirect_dma_start calls issued before the gathers.
# Measured NET NEGATIVE (default 0): the mlp IRAM overlay load blocks the
# whole Pool engine, so these cannot actually run during the overlay
# window -- they just serialize with the gathers (~1.1us per call).
IND_CALLS = int(os.environ.get("MP_RP_IND", "0"))


def _runs_pack(keep_row):
    """Greedy {4,2,1} run packing -> {g: start rows}, exact cover of kept."""
    t = np.flatnonzero(keep_row)
    out = {g: np.empty(0, np.int32) for g in GRANS}
    if len(t) == 0:
        return out
    new_run = np.ones(len(t), dtype=bool)
    new_run[1:] = np.diff(t) > 1
    run_id = np.cumsum(new_run) - 1
    run_start_pos = np.flatnonzero(new_run)
    pos = np.arange(len(t)) - run_start_pos[run_id]
    rl = np.bincount(run_id)[run_id]
    out[4] = t[(pos % 4 == 0) & (pos + 4 <= rl)].astype(np.int32)
    out[2] = t[(pos % 4 == 0) & (pos + 4 > rl) & (pos + 2 <= rl)].astype(np.int32)
    out[1] = t[(pos + 1 == rl) & (pos % 2 == 0)].astype(np.int32)
    return out


def _tile_split(n, cpt, small_last=False):
    """Chunk-tile sizes: full tiles first, then remainder (plus a final
    1-chunk tile when small_last, to shrink the end-of-pipeline drain)."""
    if n <= 0:
        return []
    if small_last:
        if n == 1:
            return [1]
        tiles = _tile_split(n - 1, cpt)
        tiles.append(1)
        return tiles
    tiles = [cpt] * (n // cpt)
    if n % cpt:
        tiles.append(n % cpt)
    return tiles


def _wrap16(vals):
    """[m] int -> [128, m/16] int16 wrapped (j -> partition j%16, col j//16)."""
    m = len(vals)
    w = vals.astype(np.int16).reshape(m // 16, 16).T
    return np.tile(w, (P // 16, 1))


def _balance_examples(keep):
    """Assign 4 examples per core, equalizing descriptor counts.  Returns
    assign[i] = list of 4 global example ids (largest-desc example first,
    so the dense prefix bites into the biggest one)."""
    ndesc = np.array(
        [sum(len(v) for v in _runs_pack(keep[b]).values()) for b in range(B)]
    )
    order = np.argsort(-ndesc)
    loads = [0.0] * N_CORES
    assign = [[] for _ in range(N_CORES)]
    for b in order:
        i = min(
            (i for i in range(N_CORES) if len(assign[i]) < BS),
            key=lambda i: loads[i],
        )
        assign[i].append(int(b))
        loads[i] += ndesc[b]
    return assign


def prep_runpack_inputs(x: np.ndarray, mask: np.ndarray):
    keep = ~mask
    nch_pre = PREFIX_CHUNKS
    rpre = nch_pre * P  # dense-prefix rows (from example 0 of each core)
    assert rpre <= T

    assign = _balance_examples(keep)

    core_slots = [dict() for _ in range(N_CORES)]  # core -> g -> starts
    core_exid = [dict() for _ in range(N_CORES)]
    core_counts = []
    for i in range(N_CORES):
        acc = {g: [] for g in GRANS}
        exa = {g: [] for g in GRANS}
        counts = []
        for j in range(BS):
            b = assign[i][j]
            krow = keep[b]
            counts.append(np.count_nonzero(krow))
            if j < PREFIX_EXAMPLES and rpre:
                krow = krow.copy()
                krow[:rpre] = False  # covered by the dense prefix
            packed = _runs_pack(krow)
            for g in GRANS:
                acc[g].append(packed[g] + j * T)
                exa[g].append(np.full(len(packed[g]), j, np.int32))
        for g in GRANS:
            core_slots[i][g] = np.concatenate(acc[g])
            core_exid[i][g] = np.concatenate(exa[g])
        core_counts.append(counts)

    # pull the first nind*128 singles out for the indirect-DMA prologue
    nind = min(IND_CALLS, min(len(core_slots[i][1]) for i in range(N_CORES)) // P)
    core_ioff = []
    core_iex = []
    for i in range(N_CORES):
        m = nind * P
        core_ioff.append(core_slots[i][1][:m].astype(np.int32))
        core_iex.append(core_exid[i][1][:m])
        core_slots[i][1] = core_slots[i][1][m:]
        core_exid[i][1] = core_exid[i][1][m:]

    # uniform chunk counts across cores (one program for all)
    CG = {
        g: max((len(core_slots[i][g]) + P - 1) // P for i in range(N_CORES))
        for g in GRANS
    }
    # program order: big descriptors first (they overfeed the SDMA engines,
    # singles underfeed -- this ordering drains the ring backlog by the end),
    # a 1-chunk singles tile last to shrink the final drain
    order = []
    for g in GRANS:
        for tc in _tile_split(CG[g], G_CPT[g], small_last=(g == 1)):
            order.append((g, tc))

    # weight columns:
    # [indirect calls][dense prefix chunks x examples][gran 4][gran 2][gran 1]
    woff = {}
    o = nind + nch_pre * PREFIX_EXAMPLES
    for g in GRANS:
        woff[g] = o
        o += g * CG[g]
    nwcols = o

    in_maps = []
    for i in range(N_CORES):
        inv = 1.0 / np.asarray(core_counts[i], np.float32)
        wts = np.zeros((P, nwcols, BS), dtype=np.float32)
        # indirect-call weights: call c row p -> col c
        s = np.arange(nind * P)
        wts[s % P, s // P, core_iex[i]] = inv[core_iex[i]]
        # dense prefix weights: row p*nch_pre + n of example j -> col nind+j*nch_pre+n
        if rpre:
            for j in range(PREFIX_EXAMPLES):
                kp = keep[assign[i][j], :rpre].reshape(P, nch_pre)
                wts[:, nind + j * nch_pre : nind + (j + 1) * nch_pre, j] = kp * inv[j]
        slots_p = {}
        for g in GRANS:
            slots = core_slots[i][g]
            ex = core_exid[i][g]
            n = len(slots)
            slots_p[g] = np.concatenate(
                [slots, np.zeros(CG[g] * P - n, dtype=np.int32)]
            )
            s = np.arange(n)
            for h in range(g):
                wts[s % P, woff[g] + g * (s // P) + h, ex] = inv[ex]

        # idx tensor: per-tile wrapped segments in program order
        segs = []
        cur = {g: 0 for g in GRANS}
        for g, tc in order:
            m = tc * P
            segs.append(_wrap16(slots_p[g][cur[g] * P : cur[g] * P + m]))
            cur[g] += tc
        idx128 = np.concatenate(segs, axis=1)

        im = {
            "x": None,
            "idx": np.ascontiguousarray(idx128),
            "wts": np.ascontiguousarray(wts),
        }
        if nind:
            im["ioff"] = np.ascontiguousarray(core_ioff[i].reshape(nind, P).T)
        in_maps.append(im)
    return in_maps, (CG, woff, order, nind), assign


def build_runpack_bass(meta, n_cores=N_CORES):
    CG, woff, order, nind = meta
    nch_pre = PREFIX_CHUNKS
    npre_ex = PREFIX_EXAMPLES
    nidxcols = sum(CG[g] for g in GRANS) * P // 16
    nwcols = nind + nch_pre * npre_ex + sum(g * CG[g] for g in GRANS)
    total_mm = nwcols  # one matmul per weight column

    nc = bacc.Bacc(
        trn_type="TRN2",
        target_bir_lowering=False,
        debug=False,
        num_devices=n_cores,
    )
    x = nc.dram_tensor("x", [BS * T, D], F32R, kind="ExternalInput").ap()
    idx = nc.dram_tensor("idx", [P, nidxcols], mybir.dt.int16, kind="ExternalInput").ap()
    wts = nc.dram_tensor("wts", [P, nwcols, BS], F32R, kind="ExternalInput").ap()
    out = nc.dram_tensor("out", [BS, D], mybir.dt.float32, kind="ExternalOutput").ap()
    ioff = None
    if nind:
        ioff = nc.dram_tensor(
            "ioff", [P, nind], mybir.dt.int32, kind="ExternalInput"
        ).ap()

    # overlapping views: row i -> g*D contiguous f32 starting at row i
    xview = {}
    for g in GRANS:
        v = x.copy()
        v.ap = type(x.ap)([[D, BS * T - (g - 1)], [1, g * D]])
        xview[g] = v

    with tile.TileContext(nc) as tc, ExitStack() as ctx:
        singles_pool = ctx.enter_context(tc.tile_pool(name="singles", bufs=1))
        pools = {
            g: ctx.enter_context(tc.tile_pool(name=f"pool{g}", bufs=G_BUFS[g]))
            for g in GRANS
            if CG[g]
        }
        psum = ctx.enter_context(tc.tile_pool(name="psum", bufs=1, space="PSUM"))

        # kick the mlp IRAM overlay load as early as possible -- the first
        # dma_gather blocks on it for ~10us
        nc.gpsimd.load_library(library_config.mlp)

        idx_sb = singles_pool.tile([P, nidxcols], mybir.dt.int16)
        nc.sync.dma_start(out=idx_sb, in_=idx)
        wts_sb = singles_pool.tile([P, nwcols, BS], F32R)
        nc.sync.dma_start(out=wts_sb, in_=wts)

        acc = psum.tile([BS, D], mybir.dt.float32)

        mm = 0  # matmul counter for start/stop flags
        icol = 0  # idx column cursor (16-wrapped units)
        cur = {g: 0 for g in GRANS}  # chunk cursors

        # indirect-DMA prologue: fetch 128 kept single-rows per call through
        # the mainline SWDGE path while the mlp overlay loads.  high_priority
        # pins these to the front of the Pool-engine schedule -- without it
        # the scheduler interleaves them between gathers where they are pure
        # serial cost.
        if nind:
            with tc.high_priority():
                ioff_sb = singles_pool.tile([P, nind], mybir.dt.int32)
                nc.sync.dma_start(out=ioff_sb, in_=ioff)
                indpool = ctx.enter_context(tc.tile_pool(name="indpool", bufs=3))
                for c in range(nind):
                    xt = indpool.tile([P, D], F32R, tag="xi")
                    nc.gpsimd.indirect_dma_start(
                        out=xt[:],
                        out_offset=None,
                        in_=x,
                        in_offset=bass.IndirectOffsetOnAxis(
                            ap=ioff_sb[:, c : c + 1], axis=0
                        ),
                    )
                    nc.tensor.matmul(
                        acc,
                        wts_sb[:, c, :],
                        xt[:],
                        start=(mm == 0),
                        stop=(mm == total_mm - 1),
                    )
                    mm += 1

        # dense prefix: HWDGE dma_starts (no Q7 involvement, and they must
        # not queue on the SWDGE ring where they would delay the mlp library
        # overlay load) + keep-weighted matmuls
        pre_eng = [nc.scalar, nc.sync]
        for j in range(npre_ex if nch_pre else 0):
            xpre = singles_pool.tile([P, nch_pre, D], F32R, tag=f"xpre{j}")
            pre_eng[j % 2].dma_start(
                out=xpre,
                in_=x[j * T : j * T + nch_pre * P].rearrange(
                    "(p n) d -> p n d", p=P
                ),
            )
            for n in range(nch_pre):
                nc.tensor.matmul(
                    acc,
                    wts_sb[:, nind + j * nch_pre + n, :],
                    xpre[:, n, :],
                    start=(mm == 0),
                    stop=(mm == total_mm - 1),
                )
                mm += 1

        for g, tc_n in order:
            m = tc_n * P
            xt = pools[g].tile([P, tc_n, g * D], F32R, tag=f"x{g}")
            nc.gpsimd.dma_gather(
                xt[:],
                xview[g],
                idx_sb[:, icol : icol + m // 16],
                m,
                m,
                g * D,
                elem_step=D,
            )
            for k in range(tc_n):
                c = cur[g] + k
                for h in range(g):
                    nc.tensor.matmul(
                        acc,
                        wts_sb[:, woff[g] + g * c + h, :],
                        xt[:, k, h * D : (h + 1) * D],
                        start=(mm == 0),
                        stop=(mm == total_mm - 1),
                    )
                    mm += 1
            cur[g] += tc_n
            icol += m // 16
        assert mm == total_mm

        o_sb = singles_pool.tile([BS, D], mybir.dt.float32)
        nc.vector.tensor_copy(out=o_sb, in_=acc)
        nc.sync.dma_start(out=out, in_=o_sb)

    nc.finalize()
    return nc


def prepare_runpack(x: np.ndarray, mask: np.ndarray):
    in_maps, meta, assign = prep_runpack_inputs(x, mask)
    for i in range(N_CORES):
        in_maps[i]["x"] = np.ascontiguousarray(x[assign[i]]).reshape(BS * T, D)
    nc = build_runpack_bass(meta)

    def unshard(results):
        out = np.empty((B, D), dtype=np.float32)
        for i in range(N_CORES):
            out[assign[i]] = results[i]["out"]
        return out

    return nc, in_maps, unshard


# ---------------------------------------------------------------------------
# Indirect-DMA implementation (mainline SWDGE dynamic AP instead of the
# dma_gather extended instruction -- different Q7 descriptor-gen path).
# Layout: slot (p, c) of the [128, NCOL, 512] gathered tensor reads row
# off[p, c]; within a tile of ct columns the flat slot order is p-major.
# ---------------------------------------------------------------------------


def prep_indirect_inputs(x: np.ndarray, mask: np.ndarray):
    keep = ~mask
    cpt = G_CHUNKS_PER_TILE

    core_slots = []
    core_examples = []
    core_counts = []
    for i in range(N_CORES):
        slots = []
        exids = []
        counts = []
        for j in range(BS):
            b = i * BS + j
            idx = np.flatnonzero(keep[b])
            counts.append(len(idx))
            slots.append(idx.astype(np.int32) + j * T)
            exids.append(np.full(len(idx), j, dtype=np.int32))
        core_slots.append(np.concatenate(slots))
        core_examples.append(np.concatenate(exids))
        core_counts.append(counts)

    ncol = max((len(s) + P - 1) // P for s in core_slots)
    ntiles = (ncol + cpt - 1) // cpt
    tile_cols = [cpt] * (ntiles - 1) + [ncol - cpt * (ntiles - 1)]
    nslots = ncol * P

    in_maps = []
    for i in range(N_CORES):
        slots = core_slots[i]
        exids = core_examples[i]
        n = len(slots)
        pad = nslots - n
        slots_p = np.concatenate([slots, np.zeros(pad, dtype=np.int32)])
        exids_p = np.concatenate([exids, np.zeros(pad, dtype=np.int32)])
        inv = 1.0 / np.asarray(core_counts[i], np.float32)

        off = np.zeros((P, ncol), dtype=np.int32)
        wts = np.zeros((P, ncol, BS), dtype=np.float32)
        pos = 0
        c0 = 0
        for ct in tile_cols:
            m = ct * P
            blk = slots_p[pos : pos + m].reshape(P, ct)
            off[:, c0 : c0 + ct] = blk
            eb = exids_p[pos : pos + m].reshape(P, ct)
            pp, cc = np.meshgrid(np.arange(P), np.arange(ct), indexing="ij")
            w = np.zeros((P, ct, BS), dtype=np.float32)
            valid = (pos + np.arange(m).reshape(P, ct)) < n
            w[pp, cc, eb] = np.where(valid, inv[eb], 0.0)
            wts[:, c0 : c0 + ct, :] = w
            pos += m
            c0 += ct

        in_maps.append(
            {
                "x": None,
                "off": np.ascontiguousarray(off),
                "wts": np.ascontiguousarray(wts),
            }
        )
    return in_maps, ncol, tile_cols


def build_indirect_bass(ncol, tile_cols, x_bufs=None, n_cores=N_CORES):
    if x_bufs is None:
        x_bufs = G_X_BUFS
    nc = bacc.Bacc(
        trn_type="TRN2",
        target_bir_lowering=False,
        debug=False,
        num_devices=n_cores,
    )
    x = nc.dram_tensor("x", [BS * T, D], F32R, kind="ExternalInput").ap()
    off = nc.dram_tensor("off", [P, ncol], mybir.dt.int32, kind="ExternalInput").ap()
    wts = nc.dram_tensor("wts", [P, ncol, BS], F32R, kind="ExternalInput").ap()
    out = nc.dram_tensor("out", [BS, D], mybir.dt.float32, kind="ExternalOutput").ap()

    with tile.TileContext(nc) as tc, ExitStack() as ctx:
        singles = ctx.enter_context(tc.tile_pool(name="singles", bufs=1))
        xpool = ctx.enter_context(tc.tile_pool(name="xpool", bufs=x_bufs))
        psum = ctx.enter_context(tc.tile_pool(name="psum", bufs=1, space="PSUM"))

        off_sb = singles.tile([P, ncol], mybir.dt.int32)
        nc.sync.dma_start(out=off_sb, in_=off)
        wts_sb = singles.tile([P, ncol, BS], F32R)
        nc.sync.dma_start(out=wts_sb, in_=wts)

        acc = psum.tile([BS, D], mybir.dt.float32)

        c0 = 0
        for ct in tile_cols:
            xt = xpool.tile([P, ct, D], F32R, tag="xt")
            nc.gpsimd.indirect_dma_start(
                out=xt[:],
                out_offset=None,
                in_=x,
                in_offset=bass.IndirectOffsetOnAxis(
                    ap=off_sb[:, c0 : c0 + ct],
                    axis=0,
                ),
            )
            for k in range(ct):
                c = c0 + k
                nc.tensor.matmul(
                    acc,
                    wts_sb[:, c, :],
                    xt[:, k, :],
                    start=(c == 0),
                    stop=(c == ncol - 1),
                )
            c0 += ct

        o_sb = singles.tile([BS, D], mybir.dt.float32)
        nc.vector.tensor_copy(out=o_sb, in_=acc)
        nc.sync.dma_start(out=out, in_=o_sb)

    nc.finalize()
    return nc


def prepare_indirect(x: np.ndarray, mask: np.ndarray):
    in_maps, ncol, tile_cols = prep_indirect_inputs(x, mask)
    for i in range(N_CORES):
        in_maps[i]["x"] = np.ascontiguousarray(x[i * BS : (i + 1) * BS]).reshape(
            BS * T, D
        )
    nc = build_indirect_bass(ncol, tile_cols)
    return nc, in_maps


# ---------------------------------------------------------------------------
# Dense fallback (previous implementation)
# ---------------------------------------------------------------------------


def build_bass(
    bs=BS,
    t=T,
    d=D,
    chunks_per_tile=CHUNKS_PER_TILE,
    x_bufs=X_BUFS,
    mm_dtype=MM_DTYPE,
    n_cores=N_CORES,
    n_dma_engines=N_DMA_ENGINES,
):
    nchunk = t // P
    assert t % P == 0 and nchunk % chunks_per_tile == 0
    nc = bacc.Bacc(
        trn_type="TRN2",
        target_bir_lowering=False,
        debug=False,
        num_devices=n_cores,
    )
    mmdt = mybir.dt.float32r if mm_dtype == "f32r" else mybir.dt.float32
    x = nc.dram_tensor("x", [bs, t, d], mmdt, kind="ExternalInput").ap()
    mask = nc.dram_tensor("mask", [bs, t], mybir.dt.uint8, kind="ExternalInput").ap()
    out = nc.dram_tensor("out", [bs, d], mybir.dt.float32, kind="ExternalOutput").ap()

    with tile.TileContext(nc) as tc, ExitStack() as ctx:
        singles = ctx.enter_context(tc.tile_pool(name="singles", bufs=1))
        xpool = ctx.enter_context(tc.tile_pool(name="xpool", bufs=x_bufs))
        tails = ctx.enter_context(tc.tile_pool(name="tails", bufs=4))
        psum = ctx.enter_context(tc.tile_pool(name="psum", bufs=1, space="PSUM"))
        accs = ctx.enter_context(tc.tile_pool(name="accs", bufs=4, space="PSUM"))

        jcols = bs * nchunk
        assert jcols <= 512

        ones = singles.tile([P, 1], mmdt)
        if mmdt == mybir.dt.float32r:
            ones_f32 = singles.tile([P, 1], mybir.dt.float32)
            nc.vector.memset(ones_f32, 1.0)
            nc.vector.tensor_copy(out=ones, in_=ones_f32)
        else:
            nc.vector.memset(ones, 1.0)

        m_u8 = singles.tile([P, bs, nchunk], mybir.dt.uint8)
        nc.sync.dma_start(out=m_u8, in_=mask.rearrange("b (p n) -> p b n", p=P))
        m_f = singles.tile([P, bs, nchunk], mybir.dt.float32)
        nc.vector.tensor_copy(out=m_f, in_=m_u8)
        keep = singles.tile([P, bs, nchunk], mmdt)
        nc.vector.tensor_scalar(
            out=keep,
            in0=m_f,
            scalar1=-1.0,
            scalar2=1.0,
            op0=mybir.AluOpType.mult,
            op1=mybir.AluOpType.add,
        )

        den_ps = psum.tile([1, bs, nchunk], mybir.dt.float32)
        nc.tensor.matmul(den_ps, ones, keep, start=True, stop=True)
        den = tails.tile([1, bs], mybir.dt.float32)
        nc.vector.tensor_reduce(
            out=den,
            in_=den_ps,
            axis=mybir.AxisListType.X,
            op=mybir.AluOpType.add,
        )
        rec = tails.tile([1, bs], mybir.dt.float32)
        nc.vector.reciprocal(rec, den)

        if n_dma_engines == 0:
            dma_engines = [nc.gpsimd]
            out_dma = nc.sync
        else:
            dma_engines = [nc.sync, nc.scalar][:n_dma_engines]
            out_dma = nc.gpsimd

        def segments(b):
            return [chunks_per_tile] * (nchunk // chunks_per_tile)

        dma_i = 0
        for b in range(bs):
            x_b = x[b].rearrange("(p n) d -> p n d", p=P)
            acc_ps = accs.tile([1, d], mybir.dt.float32)
            n0 = 0
            for seg in segments(b):
                x_tile = xpool.tile([P, seg, d], mmdt, tag="x_tile")
                dma_engines[dma_i % len(dma_engines)].dma_start(
                    out=x_tile,
                    in_=x_b[:, n0 : n0 + seg, :],
                )
                dma_i += 1
                for k in range(seg):
                    n = n0 + k
                    nc.tensor.matmul(
                        acc_ps,
                        keep[:, b, n : n + 1],
                        x_tile[:, k, :],
                        start=(n == 0),
                        stop=(n == nchunk - 1),
                    )
                n0 += seg
            o_sb = tails.tile([1, d], mybir.dt.float32)
            nc.vector.tensor_scalar_mul(o_sb, acc_ps, rec[0:1, b : b + 1])
            out_dma.dma_start(out=out[b : b + 1, :], in_=o_sb)

    nc.finalize()
    return nc


def prepare_dense(x: np.ndarray, mask: np.ndarray):
    nc = build_bass()
    mask_u8 = np.ascontiguousarray(mask).view(np.uint8)
    in_maps = [
        {
            "x": np.ascontiguousarray(x[i * BS : (i + 1) * BS]),
            "mask": np.ascontiguousarray(mask_u8[i * BS : (i + 1) * BS]),
        }
        for i in range(N_CORES)
    ]
    return nc, in_maps


def _concat_unshard(results):
    return np.concatenate([r["out"] for r in results], axis=0).astype(
        np.float32, copy=False
    )


def prepare(x: np.ndarray, mask: np.ndarray):
    """Returns (nc, in_maps, unshard) -- unshard maps per-core result dicts
    to the full [B, D] output."""
    if IMPL == "gather":
        return (*prepare_gather(x, mask), _concat_unshard)
    if IMPL == "indirect":
        return (*prepare_indirect(x, mask), _concat_unshard)
    if IMPL == "runpack":
        return prepare_runpack(x, mask)
    return (*prepare_dense(x, mask), _concat_unshard)


def kernel(x: np.ndarray, mask: np.ndarray) -> np.ndarray:
    assert x.shape == (B, T, D) and mask.shape == (B, T)
    nc, in_maps, unshard = prepare(x, mask)
    res = bass_utils.run_bass_kernel_spmd(nc, in_maps, core_ids=list(range(N_CORES)))
    return unshard(res.results).astype(np.float32, copy=False)


# revision 45
# speedup vs baseline: 1.0873x; 1.0716x over previous
"""Masked mean-pooling (nn_MaskedPooling) Trainium2 Bass kernel.

Reference semantics (jax):
    keep   = (~mask).astype(f32)               # [B, T]
    denom  = keep.sum(axis=1)                  # [B]
    out    = einsum('btd,bt->bd', x, keep) / denom[:, None]

Shapes: x [32, 4096, 512] f32, mask [32, 4096] bool -> out [32, 512] f32.

Strategy (data-parallel over batch, 8 NeuronCores, 4 examples/core):

RUN-PACKED GATHER (default, MP_IMPL=runpack), ~75 us vs 138 us for the
dense baseline:
  ~50% of the rows are masked out, so instead of streaming all of x we
  dma_gather only the kept rows out of HBM -- half the memory traffic.
  Measured bottlenecks and the responses baked in here:
  * Q7 SWDGE descriptor generation costs ~10 ns/descriptor and the Pool
    engine serializes dma_gather instructions, so descriptors are the
    scarce resource: each run of consecutive kept rows is covered
    greedily with 4-row (8 KiB), 2-row and 1-row descriptors via
    overlapping source APs (elem_step=1 row) -- exact traffic, ~1.67x
    fewer descriptors than row-at-a-time.
  * A descriptor batch only starts draining when its dma_gather's
    generation finishes, so the gather stream is tiled (~0.7 us fixed
    per call vs pipeline granularity) and ordered quads -> pairs ->
    singles: big descriptors overfeed the 16 SDMA engines, singles
    underfeed, so this ordering clears the ring backlog by the end.
  * The first dma_gather blocks ~10 us on the mlp Q7 library (IRAM
    overlay) load, which stalls the whole Pool engine.  That window is
    filled with a plain HWDGE dense read of the head of each core's x
    ("dense prefix", keep-weighted so masked rows contribute 0).
  * The reduction is one long PSUM accumulation chain: for chunk c the
    stationary operand is a host-built [128, BS] weight matrix W with
    W[p, c, j] = 1/count_j if slot c*128+p belongs to example j else 0
    (0 for pad slots).  This bakes the mean divide into the matmul AND
    makes the program shape independent of the ragged structure (all
    cores run the same NEFF with different W/idx data).
  * Examples are bin-packed onto cores by descriptor count (output is
    un-permuted on the host), equalizing the per-core critical path.
  * f32r matmuls (single-pass fp32, PE 1 cyc/row) keep the PE far off
    the critical path (~28 us busy vs ~41 us of DMA drain).

Host-side prep is mask-only (index/weight tables, ~100 KB per core);
all x traffic stays on-device.

Other implementations kept for reference / fallback:
  MP_IMPL=gather   row-at-a-time dma_gather (~122 us; gen-bound)
  MP_IMPL=dense    stream all of x, keep-weighted matmuls (~117 us
                   with MP_MM_DTYPE=f32r, ~139 us exact f32)
  MP_IMPL=indirect mainline dynamic-AP gather -- multi-row offsets
                   mis-execute on HW under walrus codegen; DO NOT USE
                   (kept only as documentation of the attempt)
"""

import os
from contextlib import ExitStack

import numpy as np

import concourse.bass as bass
import concourse.mybir as mybir
import concourse.tile as tile
from concourse import bacc, bass_utils, library_config

B, T, D = 32, 4096, 512
N_CORES = 8
BS = B // N_CORES  # examples per core
P = 128  # SBUF partitions
NCHUNK = T // P  # T-chunks per example (32)

IMPL = os.environ.get("MP_IMPL", "runpack")

# --- gather tunables ---
G_CHUNKS_PER_TILE = int(os.environ.get("MP_G_CPT", "8"))  # 1024 rows / 2 MiB
G_X_BUFS = int(os.environ.get("MP_G_XBUFS", "5"))

# --- dense tunables ---
CHUNKS_PER_TILE = int(os.environ.get("MP_CHUNKS_PER_TILE", "16"))
X_BUFS = int(os.environ.get("MP_X_BUFS", "5"))
MM_DTYPE = os.environ.get("MP_MM_DTYPE", "f32r")
N_DMA_ENGINES = int(os.environ.get("MP_DMA_ENGINES", "0"))

F32R = mybir.dt.float32r


# ---------------------------------------------------------------------------
# Ragged gather implementation
# ---------------------------------------------------------------------------


def prep_gather_inputs(x: np.ndarray, mask: np.ndarray):
    """Host-side (mask-only) prep: per-core compacted indices + weights.

    Returns (in_maps, nchunks, ntiles_chunks) where ntiles_chunks is the
    per-gather chunk count list (uniform across cores so one program fits
    all).
    """
    keep = ~mask  # [B, T] bool
    cpt = G_CHUNKS_PER_TILE

    # Per-core slot lists (global row ids into the core's flat [BS*T, D] x).
    core_slots = []
    core_examples = []  # per core: slot -> local example id
    core_counts = []
    for i in range(N_CORES):
        slots = []
        exids = []
        counts = []
        for j in range(BS):
            b = i * BS + j
            idx = np.flatnonzero(keep[b])
            counts.append(len(idx))
            slots.append(idx.astype(np.int32) + j * T)
            exids.append(np.full(len(idx), j, dtype=np.int32))
        core_slots.append(np.concatenate(slots))
        core_examples.append(np.concatenate(exids))
        core_counts.append(counts)

    nchunks = max((len(s) + P - 1) // P for s in core_slots)
    ntiles = (nchunks + cpt - 1) // cpt
    tile_chunks = [cpt] * (ntiles - 1) + [nchunks - cpt * (ntiles - 1)]
    nslots = nchunks * P

    in_maps = []
    for i in range(N_CORES):
        slots = core_slots[i]
        exids = core_examples[i]
        n = len(slots)
        pad = nslots - n
        slots_p = np.concatenate([slots, np.zeros(pad, dtype=np.int32)])

        # weights [128, nchunks, BS] f32
        wts = np.zeros((P, nchunks, BS), dtype=np.float32)
        s = np.arange(n)
        wts[s % P, s // P, exids] = 1.0 / np.asarray(core_counts[i], np.float32)[exids]

        # indices, wrapped per gather call: within a call of m idxs, idx k
        # sits at partition k%16, column k//16; replicated across the 8
        # groups of 16 partitions.  One [128, nslots//16] tensor, tiles
        # concatenated along the free dim.
        idx_cols = []
        pos = 0
        for tc in tile_chunks:
            m = tc * P
            seg = slots_p[pos : pos + m].astype(np.int16)
            idx_cols.append(seg.reshape(m // 16, 16).T)  # [16, m/16]
            pos += m
        idx16 = np.concatenate(idx_cols, axis=1)  # [16, nslots/16]
        idx128 = np.tile(idx16, (P // 16, 1))  # [128, nslots/16]

        in_maps.append(
            {
                "x": None,  # filled by caller (needs x slice)
                "idx": np.ascontiguousarray(idx128),
                "wts": np.ascontiguousarray(wts),
            }
        )
    return in_maps, nchunks, tile_chunks


def build_gather_bass(nchunks, tile_chunks, x_bufs=None, n_cores=N_CORES):
    if x_bufs is None:
        x_bufs = G_X_BUFS
    nslots = nchunks * P
    nc = bacc.Bacc(
        trn_type="TRN2",
        target_bir_lowering=False,
        debug=False,
        num_devices=n_cores,
    )
    x = nc.dram_tensor("x", [BS * T, D], F32R, kind="ExternalInput").ap()
    idx = nc.dram_tensor("idx", [P, nslots // 16], mybir.dt.int16, kind="ExternalInput").ap()
    wts = nc.dram_tensor("wts", [P, nchunks, BS], F32R, kind="ExternalInput").ap()
    out = nc.dram_tensor("out", [BS, D], mybir.dt.float32, kind="ExternalOutput").ap()

    with tile.TileContext(nc) as tc, ExitStack() as ctx:
        singles = ctx.enter_context(tc.tile_pool(name="singles", bufs=1))
        xpool = ctx.enter_context(tc.tile_pool(name="xpool", bufs=x_bufs))
        psum = ctx.enter_context(tc.tile_pool(name="psum", bufs=1, space="PSUM"))

        idx_sb = singles.tile([P, nslots // 16], mybir.dt.int16)
        nc.sync.dma_start(out=idx_sb, in_=idx)
        wts_sb = singles.tile([P, nchunks, BS], F32R)
        nc.sync.dma_start(out=wts_sb, in_=wts)

        acc = psum.tile([BS, D], mybir.dt.float32)

        c0 = 0  # chunk cursor
        s16 = 0  # idx column cursor
        for tcnt in tile_chunks:
            m = tcnt * P
            xt = xpool.tile([P, tcnt, D], F32R, tag="xt")
            nc.gpsimd.dma_gather(
                xt[:],
                x,
                idx_sb[:, s16 : s16 + m // 16],
                m,
                m,
                D,
            )
            for k in range(tcnt):
                c = c0 + k
                nc.tensor.matmul(
                    acc,
                    wts_sb[:, c, :],
                    xt[:, k, :],
                    start=(c == 0),
                    stop=(c == nchunks - 1),
                )
            c0 += tcnt
            s16 += m // 16

        o_sb = singles.tile([BS, D], mybir.dt.float32)
        nc.vector.tensor_copy(out=o_sb, in_=acc)
        nc.sync.dma_start(out=out, in_=o_sb)

    nc.finalize()
    return nc


def prepare_gather(x: np.ndarray, mask: np.ndarray):
    """Full prep for the gather impl: (nc, in_maps)."""
    in_maps, nchunks, tile_chunks = prep_gather_inputs(x, mask)
    for i in range(N_CORES):
        in_maps[i]["x"] = np.ascontiguousarray(x[i * BS : (i + 1) * BS]).reshape(
            BS * T, D
        )
    nc = build_gather_bass(nchunks, tile_chunks)
    return nc, in_maps


# ---------------------------------------------------------------------------
# Run-packed gather: cover each run of consecutive kept rows with 4 KiB
# pair-descriptors (elem_size=2 rows, elem_step=1 row -- overlapping source
# AP) plus 2 KiB single-descriptors for odd-run tails.  Exact HBM traffic
# (kept bytes only) with ~1.5x fewer descriptors than row-gather, which
# matters because Q7 SWDGE descriptor generation (~9.4 ns/idx) is the
# critical path of the row-gather kernel.
#
# Weight layout (stationary operands, [128, 2*CP + CS, BS] f32):
#   pair chunk c: col 2c weights the first row, col 2c+1 the second row.
#   single chunk c: col 2*CP + c.
# All matmuls accumulate into one [BS, D] PSUM chain; pad slots (dup row 0)
# carry weight 0.
# ---------------------------------------------------------------------------


GRANS = [4, 2, 1]  # rows per descriptor, packed greedily per run
G_CPT = {4: 4, 2: 8, 1: 8}  # chunks per gather tile
_bufs = os.environ.get("MP_RP_BUFS", "1,2,4").split(",")
G_BUFS = {4: int(_bufs[0]), 2: int(_bufs[1]), 1: int(_bufs[2])}
# The head (128*PREFIX_CHUNKS rows) of the first PREFIX_EXAMPLES examples of
# each core is read with plain HWDGE dma_starts + keep-weighted matmuls
# instead of gathers.  Those DMAs need no Q7 involvement, so they stream
# during the ~10us mlp library overlay load that blocks the first
# dma_gather -- free work in an otherwise idle window -- and they shrink
# the gather descriptor count.
PREFIX_CHUNKS = int(os.environ.get("MP_RP_PREFIX", "14"))
PREFIX_EXAMPLES = int(os.environ.get("MP_RP_PREFIX_EX", "1"))
# [128,1]-offset indirect_dma_start calls issued before the gathers.
# Measured NET NEGATIVE (default 0): the mlp IRAM overlay load blocks the
# whole Pool engine, so these cannot actually run during the overlay
# window -- they just serialize with the gathers (~1.1us per call).
IND_CALLS = int(os.environ.get("MP_RP_IND", "0"))


def _runs_pack(keep_row):
    """Greedy {4,2,1} run packing -> {g: start rows}, exact cover of kept."""
    t = np.flatnonzero(keep_row)
    out = {g: np.empty(0, np.int32) for g in GRANS}
    if len(t) == 0:
        return out
    new_run = np.ones(len(t), dtype=bool)
    new_run[1:] = np.diff(t) > 1
    run_id = np.cumsum(new_run) - 1
    run_start_pos = np.flatnonzero(new_run)
    pos = np.arange(len(t)) - run_start_pos[run_id]
    rl = np.bincount(run_id)[run_id]
    out[4] = t[(pos % 4 == 0) & (pos + 4 <= rl)].astype(np.int32)
    out[2] = t[(pos % 4 == 0) & (pos + 4 > rl) & (pos + 2 <= rl)].astype(np.int32)
    out[1] = t[(pos + 1 == rl) & (pos % 2 == 0)].astype(np.int32)
    return out


def _tile_split(n, cpt, small_last=False):
    """Chunk-tile sizes: full tiles first, then remainder (plus a final
    1-chunk tile when small_last, to shrink the end-of-pipeline drain)."""
    if n <= 0:
        return []
    if small_last:
        if n == 1:
            return [1]
        tiles = _tile_split(n - 1, cpt)
        tiles.append(1)
        return tiles
    tiles = [cpt] * (n // cpt)
    if n % cpt:
        tiles.append(n % cpt)
    return tiles


def _wrap16(vals):
    """[m] int -> [128, m/16] int16 wrapped (j -> partition j%16, col j//16)."""
    m = len(vals)
    w = vals.astype(np.int16).reshape(m // 16, 16).T
    return np.tile(w, (P // 16, 1))


def _balance_examples(keep):
    """Assign 4 examples per core, equalizing descriptor counts.  Returns
    assign[i] = list of 4 global example ids (largest-desc example first,
    so the dense prefix bites into the biggest one)."""
    ndesc = np.array(
        [sum(len(v) for v in _runs_pack(keep[b]).values()) for b in range(B)]
    )
    order = np.argsort(-ndesc)
    loads = [0.0] * N_CORES
    assign = [[] for _ in range(N_CORES)]
    for b in order:
        i = min(
            (i for i in range(N_CORES) if len(assign[i]) < BS),
            key=lambda i: loads[i],
        )
        assign[i].append(int(b))
        loads[i] += ndesc[b]
    return assign


def prep_runpack_inputs(x: np.ndarray, mask: np.ndarray):
    keep = ~mask
    nch_pre = PREFIX_CHUNKS
    rpre = nch_pre * P  # dense-prefix rows (from example 0 of each core)
    assert rpre <= T

    assign = _balance_examples(keep)

    core_slots = [dict() for _ in range(N_CORES)]  # core -> g -> starts
    core_exid = [dict() for _ in range(N_CORES)]
    core_counts = []
    for i in range(N_CORES):
        acc = {g: [] for g in GRANS}
        exa = {g: [] for g in GRANS}
        counts = []
        for j in range(BS):
            b = assign[i][j]
            krow = keep[b]
            counts.append(np.count_nonzero(krow))
            if j < PREFIX_EXAMPLES and rpre:
                krow = krow.copy()
                krow[:rpre] = False  # covered by the dense prefix
            packed = _runs_pack(krow)
            for g in GRANS:
                acc[g].append(packed[g] + j * T)
                exa[g].append(np.full(len(packed[g]), j, np.int32))
        for g in GRANS:
            core_slots[i][g] = np.concatenate(acc[g])
            core_exid[i][g] = np.concatenate(exa[g])
        core_counts.append(counts)

    # pull the first nind*128 singles out for the indirect-DMA prologue
    nind = min(IND_CALLS, min(len(core_slots[i][1]) for i in range(N_CORES)) // P)
    core_ioff = []
    core_iex = []
    for i in range(N_CORES):
        m = nind * P
        core_ioff.append(core_slots[i][1][:m].astype(np.int32))
        core_iex.append(core_exid[i][1][:m])
        core_slots[i][1] = core_slots[i][1][m:]
        core_exid[i][1] = core_exid[i][1][m:]

    # uniform chunk counts across cores (one program for all)
    CG = {
        g: max((len(core_slots[i][g]) + P - 1) // P for i in range(N_CORES))
        for g in GRANS
    }
    # program order: big descriptors first (they overfeed the SDMA engines,
    # singles underfeed -- this ordering drains the ring backlog by the end),
    # a 1-chunk singles tile last to shrink the final drain
    order = []
    for g in GRANS:
        for tc in _tile_split(CG[g], G_CPT[g], small_last=(g == 1)):
            order.append((g, tc))

    # weight columns:
    # [indirect calls][dense prefix chunks x examples][gran 4][gran 2][gran 1]
    woff = {}
    o = nind + nch_pre * PREFIX_EXAMPLES
    for g in GRANS:
        woff[g] = o
        o += g * CG[g]
    nwcols = o

    in_maps = []
    for i in range(N_CORES):
        inv = 1.0 / np.asarray(core_counts[i], np.float32)
        wts = np.zeros((P, nwcols, BS), dtype=np.float32)
        # indirect-call weights: call c row p -> col c
        s = np.arange(nind * P)
        wts[s % P, s // P, core_iex[i]] = inv[core_iex[i]]
        # dense prefix weights: row p*nch_pre + n of example j -> col nind+j*nch_pre+n
        if rpre:
            for j in range(PREFIX_EXAMPLES):
                kp = keep[assign[i][j], :rpre].reshape(P, nch_pre)
                wts[:, nind + j * nch_pre : nind + (j + 1) * nch_pre, j] = kp * inv[j]
        slots_p = {}
        for g in GRANS:
            slots = core_slots[i][g]
            ex = core_exid[i][g]
            n = len(slots)
            slots_p[g] = np.concatenate(
                [slots, np.zeros(CG[g] * P - n, dtype=np.int32)]
            )
            s = np.arange(n)
            for h in range(g):
                wts[s % P, woff[g] + g * (s // P) + h, ex] = inv[ex]

        # idx tensor: per-tile wrapped segments in program order
        segs = []
        cur = {g: 0 for g in GRANS}
        for g, tc in order:
            m = tc * P
            segs.append(_wrap16(slots_p[g][cur[g] * P : cur[g] * P + m]))
            cur[g] += tc
        idx128 = np.concatenate(segs, axis=1)

        im = {
            "x": None,
            "idx": np.ascontiguousarray(idx128),
            "wts": np.ascontiguousarray(wts),
        }
        if nind:
            im["ioff"] = np.ascontiguousarray(core_ioff[i].reshape(nind, P).T)
        in_maps.append(im)
    return in_maps, (CG, woff, order, nind), assign


def build_runpack_bass(meta, n_cores=N_CORES):
    CG, woff, order, nind = meta
    nch_pre = PREFIX_CHUNKS
    npre_ex = PREFIX_EXAMPLES
    nidxcols = sum(CG[g] for g in GRANS) * P // 16
    nwcols = nind + nch_pre * npre_ex + sum(g * CG[g] for g in GRANS)
    total_mm = nwcols  # one matmul per weight column

    nc = bacc.Bacc(
        trn_type="TRN2",
        target_bir_lowering=False,
        debug=False,
        num_devices=n_cores,
    )
    x = nc.dram_tensor("x", [BS * T, D], F32R, kind="ExternalInput").ap()
    idx = nc.dram_tensor("idx", [P, nidxcols], mybir.dt.int16, kind="ExternalInput").ap()
    wts = nc.dram_tensor("wts", [P, nwcols, BS], F32R, kind="ExternalInput").ap()
    out = nc.dram_tensor("out", [BS, D], mybir.dt.float32, kind="ExternalOutput").ap()
    ioff = None
    if nind:
        ioff = nc.dram_tensor(
            "ioff", [P, nind], mybir.dt.int32, kind="ExternalInput"
        ).ap()

    # overlapping views: row i -> g*D contiguous f32 starting at row i
    xview = {}
    for g in GRANS:
        v = x.copy()
        v.ap = type(x.ap)([[D, BS * T - (g - 1)], [1, g * D]])
        xview[g] = v

    with tile.TileContext(nc) as tc, ExitStack() as ctx:
        singles_pool = ctx.enter_context(tc.tile_pool(name="singles", bufs=1))
        pools = {
            g: ctx.enter_context(tc.tile_pool(name=f"pool{g}", bufs=G_BUFS[g]))
            for g in GRANS
            if CG[g]
        }
        psum = ctx.enter_context(tc.tile_pool(name="psum", bufs=1, space="PSUM"))

        # kick the mlp IRAM overlay load as early as possible -- the first
        # dma_gather blocks on it for ~10us
        nc.gpsimd.load_library(library_config.mlp)

        idx_sb = singles_pool.tile([P, nidxcols], mybir.dt.int16)
        nc.sync.dma_start(out=idx_sb, in_=idx)
        wts_sb = singles_pool.tile([P, nwcols, BS], F32R)
        nc.sync.dma_start(out=wts_sb, in_=wts)

        acc = psum.tile([BS, D], mybir.dt.float32)

        mm = 0  # matmul counter for start/stop flags
        icol = 0  # idx column cursor (16-wrapped units)
        cur = {g: 0 for g in GRANS}  # chunk cursors

        # indirect-DMA prologue: fetch 128 kept single-rows per call through
        # the mainline SWDGE path while the mlp overlay loads.  high_priority
        # pins these to the front of the Pool-engine schedule -- without it
        # the scheduler interleaves them between gathers where they are pure
        # serial cost.
        if nind:
            with tc.high_priority():
                ioff_sb = singles_pool.tile([P, nind], mybir.dt.int32)
                nc.sync.dma_start(out=ioff_sb, in_=ioff)
                indpool = ctx.enter_context(tc.tile_pool(name="indpool", bufs=3))
                for c in range(nind):
                    xt = indpool.tile([P, D], F32R, tag="xi")
                    nc.gpsimd.indirect_dma_start(
                        out=xt[:],
                        out_offset=None,
                        in_=x,
                        in_offset=bass.IndirectOffsetOnAxis(
                            ap=ioff_sb[:, c : c + 1], axis=0
                        ),
                    )
                    nc.tensor.matmul(
                        acc,
                        wts_sb[:, c, :],
                        xt[:],
                        start=(mm == 0),
                        stop=(mm == total_mm - 1),
                    )
                    mm += 1

        # dense prefix: HWDGE dma_starts (no Q7 involvement, and they must
        # not queue on the SWDGE ring where they would delay the mlp library
        # overlay load) + keep-weighted matmuls
        pre_eng = [nc.scalar, nc.sync]
        for j in range(npre_ex if nch_pre else 0):
            xpre = singles_pool.tile([P, nch_pre, D], F32R, tag=f"xpre{j}")
            pre_eng[j % 2].dma_start(
                out=xpre,
                in_=x[j * T : j * T + nch_pre * P].rearrange(
                    "(p n) d -> p n d", p=P
                ),
            )
            for n in range(nch_pre):
                nc.tensor.matmul(
                    acc,
                    wts_sb[:, nind + j * nch_pre + n, :],
                    xpre[:, n, :],
                    start=(mm == 0),
                    stop=(mm == total_mm - 1),
                )
                mm += 1

        for g, tc_n in order:
            m = tc_n * P
            xt = pools[g].tile([P, tc_n, g * D], F32R, tag=f"x{g}")
            nc.gpsimd.dma_gather(
                xt[:],
                xview[g],
                idx_sb[:, icol : icol + m // 16],
                m,
                m,
                g * D,
                elem_step=D,
            )
            for k in range(tc_n):
                c = cur[g] + k
                for h in range(g):
                    nc.tensor.matmul(
                        acc,
                        wts_sb[:, woff[g] + g * c + h, :],
                        xt[:, k, h * D : (h + 1) * D],
                        start=(mm == 0),
                        stop=(mm == total_mm - 1),
                    )
                    mm += 1
            cur[g] += tc_n
            icol += m // 16
        assert mm == total_mm

        o_sb = singles_pool.tile([BS, D], mybir.dt.float32)
        nc.vector.tensor_copy(out=o_sb, in_=acc)
        nc.sync.dma_start(out=out, in_=o_sb)

    nc.finalize()
    return nc


def prepare_runpack(x: np.ndarray, mask: np.ndarray):
    in_maps, meta, assign = prep_runpack_inputs(x, mask)
    for i in range(N_CORES):
        in_maps[i]["x"] = np.ascontiguousarray(x[assign[i]]).reshape(BS * T, D)
    nc = build_runpack_bass(meta)

    def unshard(results):
        out = np.empty((B, D), dtype=np.float32)
        for i in range(N_CORES):
            out[assign[i]] = results[i]["out"]
        return out

    return nc, in_maps, unshard


# ---------------------------------------------------------------------------
# Indirect-DMA implementation (mainline SWDGE dynamic AP instead of the
# dma_gather extended instruction -- different Q7 descriptor-gen path).
# Layout: slot (p, c) of the [128, NCOL, 512] gathered tensor reads row
# off[p, c]; within a tile of ct columns the flat slot order is p-major.
# ---------------------------------------------------------------------------


def prep_indirect_inputs(x: np.ndarray, mask: np.ndarray):
    keep = ~mask
    cpt = G_CHUNKS_PER_TILE

    core_slots = []
    core_examples = []
    core_counts = []
    for i in range(N_CORES):
        slots = []
        exids = []
        counts = []
        for j in range(BS):
            b = i * BS + j
            idx = np.flatnonzero(keep[b])
            counts.append(len(idx))
            slots.append(idx.astype(np.int32) + j * T)
            exids.append(np.full(len(idx), j, dtype=np.int32))
        core_slots.append(np.concatenate(slots))
        core_examples.append(np.concatenate(exids))
        core_counts.append(counts)

    ncol = max((len(s) + P - 1) // P for s in core_slots)
    ntiles = (ncol + cpt - 1) // cpt
    tile_cols = [cpt] * (ntiles - 1) + [ncol - cpt * (ntiles - 1)]
    nslots = ncol * P

    in_maps = []
    for i in range(N_CORES):
        slots = core_slots[i]
        exids = core_examples[i]
        n = len(slots)
        pad = nslots - n
        slots_p = np.concatenate([slots, np.zeros(pad, dtype=np.int32)])
        exids_p = np.concatenate([exids, np.zeros(pad, dtype=np.int32)])
        inv = 1.0 / np.asarray(core_counts[i], np.float32)

        off = np.zeros((P, ncol), dtype=np.int32)
        wts = np.zeros((P, ncol, BS), dtype=np.float32)
        pos = 0
        c0 = 0
        for ct in tile_cols:
            m = ct * P
            blk = slots_p[pos : pos + m].reshape(P, ct)
            off[:, c0 : c0 + ct] = blk
            eb = exids_p[pos : pos + m].reshape(P, ct)
            pp, cc = np.meshgrid(np.arange(P), np.arange(ct), indexing="ij")
            w = np.zeros((P, ct, BS), dtype=np.float32)
            valid = (pos + np.arange(m).reshape(P, ct)) < n
            w[pp, cc, eb] = np.where(valid, inv[eb], 0.0)
            wts[:, c0 : c0 + ct, :] = w
            pos += m
            c0 += ct

        in_maps.append(
            {
                "x": None,
                "off": np.ascontiguousarray(off),
                "wts": np.ascontiguousarray(wts),
            }
        )
    return in_maps, ncol, tile_cols


def build_indirect_bass(ncol, tile_cols, x_bufs=None, n_cores=N_CORES):
    if x_bufs is None:
        x_bufs = G_X_BUFS
    nc = bacc.Bacc(
        trn_type="TRN2",
        target_bir_lowering=False,
        debug=False,
        num_devices=n_cores,
    )
    x = nc.dram_tensor("x", [BS * T, D], F32R, kind="ExternalInput").ap()
    off = nc.dram_tensor("off", [P, ncol], mybir.dt.int32, kind="ExternalInput").ap()
    wts = nc.dram_tensor("wts", [P, ncol, BS], F32R, kind="ExternalInput").ap()
    out = nc.dram_tensor("out", [BS, D], mybir.dt.float32, kind="ExternalOutput").ap()

    with tile.TileContext(nc) as tc, ExitStack() as ctx:
        singles = ctx.enter_context(tc.tile_pool(name="singles", bufs=1))
        xpool = ctx.enter_context(tc.tile_pool(name="xpool", bufs=x_bufs))
        psum = ctx.enter_context(tc.tile_pool(name="psum", bufs=1, space="PSUM"))

        off_sb = singles.tile([P, ncol], mybir.dt.int32)
        nc.sync.dma_start(out=off_sb, in_=off)
        wts_sb = singles.tile([P, ncol, BS], F32R)
        nc.sync.dma_start(out=wts_sb, in_=wts)

        acc = psum.tile([BS, D], mybir.dt.float32)

        c0 = 0
        for ct in tile_cols:
            xt = xpool.tile([P, ct, D], F32R, tag="xt")
            nc.gpsimd.indirect_dma_start(
                out=xt[:],
                out_offset=None,
                in_=x,
                in_offset=bass.IndirectOffsetOnAxis(
                    ap=off_sb[:, c0 : c0 + ct],
                    axis=0,
                ),
            )
            for k in range(ct):
                c = c0 + k
                nc.tensor.matmul(
                    acc,
                    wts_sb[:, c, :],
                    xt[:, k, :],
                    start=(c == 0),
                    stop=(c == ncol - 1),
                )
            c0 += ct

        o_sb = singles.tile([BS, D], mybir.dt.float32)
        nc.vector.tensor_copy(out=o_sb, in_=acc)
        nc.sync.dma_start(out=out, in_=o_sb)

    nc.finalize()
    return nc


def prepare_indirect(x: np.ndarray, mask: np.ndarray):
    in_maps, ncol, tile_cols = prep_indirect_inputs(x, mask)
    for i in range(N_CORES):
        in_maps[i]["x"] = np.ascontiguousarray(x[i * BS : (i + 1) * BS]).reshape(
            BS * T, D
        )
    nc = build_indirect_bass(ncol, tile_cols)
    return nc, in_maps


# ---------------------------------------------------------------------------
# Dense fallback (previous implementation)
# ---------------------------------------------------------------------------


def build_bass(
    bs=BS,
    t=T,
    d=D,
    chunks_per_tile=CHUNKS_PER_TILE,
    x_bufs=X_BUFS,
    mm_dtype=MM_DTYPE,
    n_cores=N_CORES,
    n_dma_engines=N_DMA_ENGINES,
):
    nchunk = t // P
    assert t % P == 0 and nchunk % chunks_per_tile == 0
    nc = bacc.Bacc(
        trn_type="TRN2",
        target_bir_lowering=False,
        debug=False,
        num_devices=n_cores,
    )
    mmdt = mybir.dt.float32r if mm_dtype == "f32r" else mybir.dt.float32
    x = nc.dram_tensor("x", [bs, t, d], mmdt, kind="ExternalInput").ap()
    mask = nc.dram_tensor("mask", [bs, t], mybir.dt.uint8, kind="ExternalInput").ap()
    out = nc.dram_tensor("out", [bs, d], mybir.dt.float32, kind="ExternalOutput").ap()

    with tile.TileContext(nc) as tc, ExitStack() as ctx:
        singles = ctx.enter_context(tc.tile_pool(name="singles", bufs=1))
        xpool = ctx.enter_context(tc.tile_pool(name="xpool", bufs=x_bufs))
        tails = ctx.enter_context(tc.tile_pool(name="tails", bufs=4))
        psum = ctx.enter_context(tc.tile_pool(name="psum", bufs=1, space="PSUM"))
        accs = ctx.enter_context(tc.tile_pool(name="accs", bufs=4, space="PSUM"))

        jcols = bs * nchunk
        assert jcols <= 512

        ones = singles.tile([P, 1], mmdt)
        if mmdt == mybir.dt.float32r:
            ones_f32 = singles.tile([P, 1], mybir.dt.float32)
            nc.vector.memset(ones_f32, 1.0)
            nc.vector.tensor_copy(out=ones, in_=ones_f32)
        else:
            nc.vector.memset(ones, 1.0)

        m_u8 = singles.tile([P, bs, nchunk], mybir.dt.uint8)
        nc.sync.dma_start(out=m_u8, in_=mask.rearrange("b (p n) -> p b n", p=P))
        m_f = singles.tile([P, bs, nchunk], mybir.dt.float32)
        nc.vector.tensor_copy(out=m_f, in_=m_u8)
        keep = singles.tile([P, bs, nchunk], mmdt)
        nc.vector.tensor_scalar(
            out=keep,
            in0=m_f,
            scalar1=-1.0,
            scalar2=1.0,
            op0=mybir.AluOpType.mult,
            op1=mybir.AluOpType.add,
        )

        den_ps = psum.tile([1, bs, nchunk], mybir.dt.float32)
        nc.tensor.matmul(den_ps, ones, keep, start=True, stop=True)
        den = tails.tile([1, bs], mybir.dt.float32)
        nc.vector.tensor_reduce(
            out=den,
            in_=den_ps,
            axis=mybir.AxisListType.X,
            op=mybir.AluOpType.add,
        )
        rec = tails.tile([1, bs], mybir.dt.float32)
        nc.vector.reciprocal(rec, den)

        if n_dma_engines == 0:
            dma_engines = [nc.gpsimd]
            out_dma = nc.sync
        else:
            dma_engines = [nc.sync, nc.scalar][:n_dma_engines]
            out_dma = nc.gpsimd

        def segments(b):
            return [chunks_per_tile] * (nchunk // chunks_per_tile)

        dma_i = 0
        for b in range(bs):
            x_b = x[b].rearrange("(p n) d -> p n d", p=P)
            acc_ps = accs.tile([1, d], mybir.dt.float32)
            n0 = 0
            for seg in segments(b):
                x_tile = xpool.tile([P, seg, d], mmdt, tag="x_tile")
                dma_engines[dma_i % len(dma_engines)].dma_start(
                    out=x_tile,
                    in_=x_b[:, n0 : n0 + seg, :],
                )
                dma_i += 1
                for k in range(seg):
                    n = n0 + k
                    nc.tensor.matmul(
                        acc_ps,
                        keep[:, b, n : n + 1],
                        x_tile[:, k, :],
                        start=(n == 0),
                        stop=(n == nchunk - 1),
                    )
                n0 += seg
            o_sb = tails.tile([1, d], mybir.dt.float32)
            nc.vector.tensor_scalar_mul(o_sb, acc_ps, rec[0:1, b : b + 1])
            out_dma.dma_start(out=out[b : b + 1, :], in_=o_sb)

    nc.finalize()
    return nc


def prepare_dense(x: np.ndarray, mask: np.ndarray):
    nc = build_bass()
    mask_u8 = np.ascontiguousarray(mask).view(np.uint8)
    in_maps = [
        {
            "x": np.ascontiguousarray(x[i * BS : (i + 1) * BS]),
            "mask": np.ascontiguousarray(mask_u8[i * BS : (i + 1) * BS]),
        }
        for i in range(N_CORES)
    ]
    return nc, in_maps


def _concat_unshard(results):
    return np.concatenate([r["out"] for r in results], axis=0).astype(
        np.float32, copy=False
    )


def prepare(x: np.ndarray, mask: np.ndarray):
    """Returns (nc, in_maps, unshard) -- unshard maps per-core result dicts
    to the full [B, D] output."""
    if IMPL == "gather":
        return (*prepare_gather(x, mask), _concat_unshard)
    if IMPL == "indirect":
        return (*prepare_indirect(x, mask), _concat_unshard)
    if IMPL == "runpack":
        return prepare_runpack(x, mask)
    return (*prepare_dense(x, mask), _concat_unshard)


def kernel(x: np.ndarray, mask: np.ndarray) -> np.ndarray:
    assert x.shape == (B, T, D) and mask.shape == (B, T)
    nc, in_maps, unshard = prepare(x, mask)
    res = bass_utils.run_bass_kernel_spmd(nc, in_maps, core_ids=list(range(N_CORES)))
    return unshard(res.results).astype(np.float32, copy=False)
